# revision 16
# baseline (speedup 1.0000x reference)
"""CGCNN (3x CGConv + pooled MLP head) on 8 TRN2 NeuronCores — v2.

Sharding: dst-range node sharding (core k owns nodes [k*12500,(k+1)*12500)).
Per core, the core's 12500 nodes are assigned to 98 tiles of 128 by a
balanced greedy packer so that per-(src-chunk, dst-tile) cell edge counts
are nearly equal across cells and cores (cuts slot padding to a few %).
Edges are ordered (src-chunk bucket, dst-tile); cell group counts are padded
to the max over cores (SPMD-identical program).

Key structure per conv:
  - src features: dma_gather (transpose mode) of 256B rows from the
    (AllGathered) node table, batched ~2048 idx/call to amortize fixed cost.
  - dst features: NO gather. Per tile, Y{f,s}_t = h_tile @ W{f,s}_dst is
    computed on the PE (from an SBUF-resident transposed local table), and
    the per-edge dst term is expanded with a host-precomputed one-hot
    matrix streamed from HBM: F_dst = Yf_t^T(lhsT) x oh2.
  - messages: m = sigmoid(F) * softplus(S) with native activations
    (2 scalar acts + 1 DVE mult per chunk).
  - aggregation: per 128-edge group, PE transpose of m + one-hot matmul
    (oh streamed from HBM next to oh2) accumulated per cell in PSUM.
h tables are bf16 in HBM, replicated by AllGather after conv1 and conv2.
Pooling via indicator matmul + AllReduce; small MLP head replicated fp32.
"""
import os
import numpy as np
import ml_dtypes

import concourse.bass as bass
import concourse.bacc as bacc
import concourse.tile as tile
from concourse import mybir
from concourse.bass_utils import run_bass_kernel_spmd

dt = mybir.dt
bf16 = ml_dtypes.bfloat16

N_NODES = 100000
NODE_DIM = 3
EDGE_DIM = 32
HIDDEN = 128
OUT_DIM = 3
N_GRAPHS = 64
N_CORES = 8
NL = N_NODES // N_CORES          # 12500
NTILE = (NL + 127) // 128        # 98
NLP = NTILE * 128                # 12544
NFULL = NLP * N_CORES            # 100352
NCHUNK = 4
CHUNK = NFULL // NCHUNK          # 25088 < 32768
GB = 1536                        # gather batch tile (slots)
GCALL = 768                      # max idx per dma_gather call
A_SP = 0.69219361
B_SP = 0.42078611


def _wrap16(idx):
    w = idx.reshape(-1, 16).T.astype(np.int16).copy()
    return np.tile(w, (8, 1))       # replicate across the 8 q7 cores


def _cell_targets():
    """Shared (cross-core) per-(tile,bucket) slot targets, multiples of 128.
    95 cells @512 + 3 cells @640 per bucket, large cells spread over tiles."""
    T = np.full((NTILE, NCHUNK), 512, np.int64)
    for b in range(NCHUNK):
        for j in range(3):
            T[(b * 3 + j) % NTILE, b] = 640
    return T


def _balance_tiles(deg, T):
    """deg: [NL, NCHUNK] per-node per-bucket in-degree. Pack nodes into 98
    tiles (<=128 nodes) aiming to keep per-(tile,bucket) sums under the
    shared targets T."""
    order = np.argsort(-deg.sum(1), kind="stable")
    sums = np.zeros((NTILE, NCHUNK), np.int64)
    cap = np.full(NTILE, 128, np.int64)
    tile_of = np.empty(NL, np.int64)
    Tf = T.astype(np.float64)
    for n in order:
        d = deg[n]
        if d.sum() == 0:
            t = int(np.argmax(cap))
        else:
            ns = sums + d
            over = np.maximum(ns - T, 0).sum(axis=1).astype(np.float64)
            frac = (ns / Tf).max(axis=1)
            score = over * 1e3 + frac
            score[cap <= 0] = np.inf
            t = int(np.argmin(score))
        tile_of[n] = t
        sums[t] += d
        cap[t] -= 1
    return tile_of, sums


def _prep(inputs):
    x = np.asarray(inputs["x"], np.float32)
    ei = np.asarray(inputs["edge_index"])
    ea = np.asarray(inputs["edge_attr"], np.float32)
    batch = np.asarray(inputs["batch"]).astype(np.int64)
    src, dst_g = ei[0].astype(np.int64), ei[1].astype(np.int64)

    owner = dst_g // NL
    b_edge = (src // NL) // 2          # == pad_id // CHUNK, permutation-free

    # --- balanced tile assignment per core ---
    tile_of_g = np.empty(N_NODES, np.int64)
    pos_g = np.empty(N_NODES, np.int64)     # local padded id within owner
    cnts = np.zeros((N_CORES, NCHUNK, NTILE), np.int64)
    per_core_sel = []
    Tgt = _cell_targets()
    for k in range(N_CORES):
        sel = np.nonzero(owner == k)[0]
        d_loc = dst_g[sel] - k * NL
        deg = np.zeros((NL, NCHUNK), np.int64)
        np.add.at(deg, (d_loc, b_edge[sel]), 1)
        tile_of, sums = _balance_tiles(deg, Tgt)
        cnts[k] = sums.T                     # [NCHUNK, NTILE]
        # positions within tile: stable by node id
        pos_in = np.zeros(NL, np.int64)
        for t in range(NTILE):
            nodes = np.nonzero(tile_of == t)[0]
            pos_in[nodes] = np.arange(len(nodes))
        tile_of_g[k * NL:(k + 1) * NL] = tile_of
        pos_g[k * NL:(k + 1) * NL] = tile_of * 128 + pos_in
        per_core_sel.append((sel, d_loc))

    pid_g = (np.arange(N_NODES) // NL) * NLP + pos_g   # padded global row

    ngroups = np.ceil(cnts / 128.0).astype(np.int64).max(axis=0)  # [NCHUNK, NTILE]
    schedule = []        # (b, t, off, gw)
    off = 0
    for b in range(NCHUNK):
        for t in range(NTILE):
            g = int(ngroups[b, t])
            if g == 0:
                continue
            schedule.append((b, t, off, g * 128))
            off += g * 128
    nslot = off
    maxgw = max(s[3] for s in schedule)
    assert maxgw <= 768, maxgw

    # gather batches: cell-aligned, <= GB slots, single bucket
    batches = []         # (off, n, bkt)
    cur_off, cur_n, cur_b = None, 0, None
    for (b, t, o, gw) in schedule:
        if cur_off is not None and (b != cur_b or cur_n + gw > GB):
            batches.append((cur_off, cur_n, cur_b))
            cur_off = None
        if cur_off is None:
            cur_off, cur_n, cur_b = o, 0, b
        cur_n += gw
    if cur_off is not None:
        batches.append((cur_off, cur_n, cur_b))

    # --- weights ---
    Wf1 = np.asarray(inputs["Wf1"], np.float32); bf1 = np.asarray(inputs["bf1"], np.float32)
    Ws1 = np.asarray(inputs["Ws1"], np.float32); bs1 = np.asarray(inputs["bs1"], np.float32)
    Wp = np.asarray(inputs["Wp"], np.float32); bp = np.asarray(inputs["bp"], np.float32)
    P = {nm: np.asarray(inputs[nm], np.float32) for nm in
         ["Wf2", "bf2", "Ws2", "bs2", "Wf3", "bf3", "Ws3", "bs3", "W1", "b1", "W2", "b2"]}

    def cw(W, b):
        ws = W[HIDDEN:2 * HIDDEN]                     # src part  [128,128]
        we = np.zeros((33, HIDDEN), np.float32)
        we[:32] = W[2 * HIDDEN:]
        we[32] = b
        wd = W[:HIDDEN]                               # dst part (rhs for Y)
        return ws.astype(bf16), we.astype(bf16), wd.astype(bf16)

    w2s, w2e, w2da = cw(P["Wf2"], P["bf2"]); s2s, s2e, s2da = cw(P["Ws2"], P["bs2"])
    w3s, w3e, w3da = cw(P["Wf3"], P["bf3"]); s3s, s3e, s3da = cw(P["Ws3"], P["bs3"])
    w2d = np.concatenate([w2da, s2da], axis=1)        # [128, 256]
    w3d = np.concatenate([w3da, s3da], axis=1)

    def c1w(W, b):
        ws = np.zeros((128, 4), np.float32); ws[0:3, :3] = W[3:6]
        we = np.zeros((33, 4), np.float32); we[:32, :3] = W[6:]; we[32, :3] = b
        wd = np.zeros((4, 4), np.float32); wd[0:3, :3] = W[0:3]
        return ws.astype(bf16), we.astype(bf16), wd.astype(bf16)

    w1s, w1e, w1d = c1w(Wf1, bf1); s1s, s1e, s1d = c1w(Ws1, bs1)
    wp_aug = np.zeros((4, HIDDEN), np.float32); wp_aug[:3] = Wp; wp_aug[3] = bp

    gcnts = np.bincount(batch, minlength=N_GRAPHS).astype(np.float32)
    inv_cnt = (1.0 / np.maximum(gcnts, 1.0)).reshape(N_GRAPHS, 1)

    identb = np.eye(128, dtype=np.float32).astype(bf16)
    identf = np.eye(128, dtype=np.float32)

    xpad = np.zeros((NFULL, 128), bf16)
    xpad[pid_g, :3] = x.astype(bf16)

    in_maps = []
    for k in range(N_CORES):
        sel, d_loc = per_core_sel[k]
        dslot = pos_g[k * NL:(k + 1) * NL][d_loc]          # padded local id of dst
        d_tile = dslot // 128
        d_in_tile = dslot % 128
        eorder = np.lexsort((d_in_tile, d_tile, b_edge[sel]))
        es, dt_s, dit_s, bb = (sel[eorder], d_tile[eorder],
                               d_in_tile[eorder], b_edge[sel][eorder])

        gsrc = np.zeros(nslot, np.int16)
        eaT = np.zeros((33, nslot), np.float32)
        ohblk = np.zeros((128, 2 * nslot), bf16)
        ptr = 0
        n_e = len(es)
        for (b, t, o, gw) in schedule:
            p2 = ptr
            while p2 < n_e and bb[p2] == b and dt_s[p2] == t:
                p2 += 1
            cnt = p2 - ptr
            assert cnt <= gw
            gsrc[o:o + cnt] = (pid_g[src[es[ptr:p2]]] % CHUNK).astype(np.int16)
            eaT[:32, o:o + cnt] = ea[es[ptr:p2]].T
            eaT[32, o:o + cnt] = 1.0
            dd = dit_s[ptr:p2]
            # oh2 [node_slot, edge]
            ohblk[dd, 2 * o + np.arange(cnt)] = 1.0
            # oh per group [edge_in_group, node_slot]; 0.5 folds sigmoid's /2
            jj = np.arange(cnt)
            ohblk[jj % 128, 2 * o + gw + (jj // 128) * 128 + dd] = 0.5
            ptr = p2
        assert ptr == n_e

        xT_sb = np.zeros((4, NLP), np.float32)
        xl = x[k * NL:(k + 1) * NL]
        pidl = pos_g[k * NL:(k + 1) * NL]
        xT_sb[:3, pidl] = xl.T

        xrow = np.zeros((128, NTILE * 4), np.float32)
        xrow[pidl % 128, (pidl // 128) * 4 + 0] = xl[:, 0]
        xrow[pidl % 128, (pidl // 128) * 4 + 1] = xl[:, 1]
        xrow[pidl % 128, (pidl // 128) * 4 + 2] = xl[:, 2]
        xrow[:, 3::4] = 1.0

        ind = np.zeros((NLP, N_GRAPHS), np.float32)
        ind[pidl, batch[k * NL:(k + 1) * NL]] = 1.0

        in_maps.append(dict(
            xpad=xpad,
            gsrc=_wrap16(gsrc),
            eaT=eaT.astype(bf16),
            ohblk=ohblk,
            xT_sb=xT_sb.astype(bf16),
            xrow=xrow,
            identb=identb, identf=identf,
            ind=ind.reshape(NTILE, 128, N_GRAPHS).transpose(1, 0, 2)
                  .reshape(128, NTILE * N_GRAPHS).copy(),
            inv_cnt=inv_cnt,
            w1s=w1s, w1e=w1e, w1d=w1d, s1s=s1s, s1e=s1e, s1d=s1d,
            w2s=w2s, w2e=w2e, w2d=w2d, s2s=s2s, s2e=s2e,
            w3s=w3s, w3e=w3e, w3d=w3d, s3s=s3s, s3e=s3e,
            wp_aug=wp_aug.astype(bf16),
            hw1=P["W1"], hb1=P["b1"].reshape(1, HIDDEN).copy(),
            hw2=P["W2"], hb2=P["b2"].reshape(1, OUT_DIM).copy(),
        ))
    return in_maps, schedule, batches, nslot, maxgw


def _conv_pass(nc, cdim, table_d, gidx_t, eaT_d, ohblk_d, y_src, wfs, wfe,
               wss, wse, id_t, schedule, batches, agg, pools):
    """y_src: ('sbuf', tile) with [128, NTILE*8] for conv1 (f cols 0:4, s 4:8)
    or ('hbm', dram) with [128, NTILE*256] for conv2/3 (f 0:128, s 128:256)."""
    pool, psum_fs, psum_m, psum_ag, gpool, ipool = pools
    cp = 4 if cdim == 3 else 128
    acols = 4 if cdim == 3 else 128
    A = mybir.ActivationFunctionType

    # map cell -> batch
    cell_batch = {}
    for bi, (boff, bn, bkt) in enumerate(batches):
        for ci, (b, t, o, gw) in enumerate(schedule):
            if boff <= o < boff + bn:
                cell_batch[ci] = bi

    def gather(bi):
        boff, bn, bkt = batches[bi]
        hb = gpool.tile([128, GB], dt.bfloat16, tag="hsrc")
        for j in range(0, bn, GCALL):
            n = min(GCALL, bn - j)
            nc.gpsimd.dma_gather(
                out_ap=hb[:, j:j + n].rearrange("p (g e) -> p g e", g=1),
                in_ap=table_d[bkt * CHUNK:(bkt + 1) * CHUNK, :],
                idxs_ap=gidx_t[:, (boff + j) // 16:(boff + j + n) // 16],
                num_idxs=n, num_idxs_reg=n, elem_size=128, transpose=True)
        return hb

    pending = {}
    pending[0] = gather(0)
    if len(batches) > 1:
        pending[1] = gather(1)
    cur_bi = 0
    hb = pending.pop(0)
    for ci, (b, t, o, gw) in enumerate(schedule):
        bi = cell_batch[ci]
        if bi != cur_bi:
            hb = pending.pop(bi) if bi in pending else gather(bi)
            cur_bi = bi
        for pf in (bi + 1, bi + 2):
            if pf < len(batches) and pf not in pending and pf > bi:
                if all(p <= bi or p in pending for p in range(bi + 1, pf)):
                    if pf - bi <= 2 and pf not in pending:
                        pending[pf] = gather(pf)

        boff = batches[bi][0]
        oh_t = ipool.tile([128, 2 * 768], dt.bfloat16, tag="oh")
        nc.sync.dma_start(out=oh_t[:, :2 * gw], in_=ohblk_d[:, 2 * o:2 * o + 2 * gw])
        ea_t = ipool.tile([33, 768], dt.bfloat16, tag="ea")
        nc.sync.dma_start(out=ea_t[:, :gw], in_=eaT_d[:, o:o + gw])
        if y_src[0] == "hbm":
            y_t = ipool.tile([128, 256], dt.bfloat16, tag="yy")
            nc.sync.dma_start(out=y_t[:], in_=y_src[1][:, t * 256:(t + 1) * 256])
            yf, ys = y_t[:, 0:cdim], y_t[:, 128:128 + cdim]
        else:
            y_t = y_src[1]
            yf, ys = y_t[:, t * 8:t * 8 + 4], y_t[:, t * 8 + 4:t * 8 + 8]

        ng = gw // 128
        ag = psum_ag.tile([128, acols], dt.float32, space="PSUM", tag="ag",
                          name=f"ag{ci}")
        gg = 0
        for c0 in range(0, ng, 4):
            cn = min(4, ng - c0)
            cw_ = cn * 128
            co = c0 * 128
            f_ps = psum_fs.tile([128, 512], dt.float32, space="PSUM", tag="f")
            s_ps = psum_fs.tile([128, 512], dt.float32, space="PSUM", tag="s")
            hs = hb[:, o - boff + co:o - boff + co + cw_]
            eb = ea_t[:33, co:co + cw_]
            o2 = oh_t[:, co:co + cw_]
            nc.tensor.matmul(f_ps[:cp, :cw_], lhsT=wfs[:], rhs=hs, start=True, stop=False)
            nc.tensor.matmul(f_ps[:cp, :cw_], lhsT=wfe[:], rhs=eb, start=False, stop=False)
            nc.tensor.matmul(f_ps[:cp, :cw_], lhsT=yf, rhs=o2, start=False, stop=True)
            nc.tensor.matmul(s_ps[:cp, :cw_], lhsT=wss[:], rhs=hs, start=True, stop=False)
            nc.tensor.matmul(s_ps[:cp, :cw_], lhsT=wse[:], rhs=eb, start=False, stop=False)
            nc.tensor.matmul(s_ps[:cp, :cw_], lhsT=ys, rhs=o2, start=False, stop=True)

            # m' = (tanh(F/2)+1) * (silu(S) + A - A*tanh^2(B*S));  the /2 of
            # sigmoid is folded into the one-hot aggregation values (0.5)
            O = mybir.AluOpType
            tf = pool.tile([128, 512], dt.bfloat16, tag="tf")
            ss = pool.tile([128, 512], dt.bfloat16, tag="ss")
            ts = pool.tile([128, 512], dt.bfloat16, tag="ts")
            nc.scalar.activation(tf[:cp, :cw_], f_ps[:cp, :cw_], A.Tanh, scale=0.5)
            nc.scalar.activation(ss[:cp, :cw_], s_ps[:cp, :cw_], A.Silu)
            nc.scalar.activation(ts[:cp, :cw_], s_ps[:cp, :cw_], A.Tanh, scale=B_SP)
            t1 = pool.tile([128, 512], dt.bfloat16, tag="t1")
            nc.vector.scalar_tensor_tensor(out=t1[:cp, :cw_], in0=ts[:cp, :cw_],
                                           scalar=-A_SP, in1=ts[:cp, :cw_],
                                           op0=O.mult, op1=O.mult)
            t2 = pool.tile([128, 512], dt.bfloat16, tag="t2")
            nc.vector.scalar_tensor_tensor(out=t2[:cp, :cw_], in0=t1[:cp, :cw_],
                                           scalar=A_SP, in1=ss[:cp, :cw_],
                                           op0=O.add, op1=O.add)
            m_bf = pool.tile([128, 512], dt.bfloat16, tag="mbf")
            nc.vector.scalar_tensor_tensor(out=m_bf[:cp, :cw_], in0=tf[:cp, :cw_],
                                           scalar=1.0, in1=t2[:cp, :cw_],
                                           op0=O.add, op1=O.mult)

            for g in range(cn):
                m_ps = psum_m.tile([128, 128], dt.bfloat16, space="PSUM", tag="mt")
                nc.tensor.transpose(m_ps[:, :cdim], m_bf[:cdim, g * 128:(g + 1) * 128],
                                    id_t[:cdim, :cdim])
                m_sb = pool.tile([128, 128], dt.bfloat16, tag="msb")
                if (gg % 2) == 0:
                    nc.vector.tensor_copy(out=m_sb[:, :cdim], in_=m_ps[:, :cdim])
                else:
                    nc.scalar.copy(out=m_sb[:, :cdim], in_=m_ps[:, :cdim])
                ohg = oh_t[:, gw + (gg * 128):gw + (gg + 1) * 128]
                nc.tensor.matmul(ag[:, :cdim], lhsT=ohg, rhs=m_sb[:, :cdim],
                                 start=(gg == 0), stop=(gg == ng - 1))
                gg += 1
        nc.vector.tensor_add(out=agg[:, t * acols:t * acols + cdim],
                             in0=agg[:, t * acols:t * acols + cdim],
                             in1=ag[:, :cdim])


def build(schedule, batches, nslot, maxgw):
    nc = bacc.Bacc("TRN2", target_bir_lowering=False, debug=False, num_devices=N_CORES)
    D = {}

    def din(name, shape, dtype):
        D[name] = nc.dram_tensor(name, list(shape), dtype, kind="ExternalInput")
        return D[name]

    xpad_d = din("xpad", (NFULL, 128), dt.bfloat16)
    gsrc_d = din("gsrc", (128, nslot // 16), dt.int16)
    eaT_d = din("eaT", (33, nslot), dt.bfloat16)
    ohblk_d = din("ohblk", (128, 2 * nslot), dt.bfloat16)
    xT_d = din("xT_sb", (4, NLP), dt.bfloat16)
    xrow_d = din("xrow", (128, NTILE * 4), dt.float32)
    identb_d = din("identb", (128, 128), dt.bfloat16)
    identf_d = din("identf", (128, 128), dt.float32)
    ind_d = din("ind", (128, NTILE * N_GRAPHS), dt.float32)
    invc_d = din("inv_cnt", (N_GRAPHS, 1), dt.float32)
    wshapes = [("w1s", (128, 4)), ("w1e", (33, 4)), ("w1d", (4, 4)),
               ("s1s", (128, 4)), ("s1e", (33, 4)), ("s1d", (4, 4)),
               ("w2s", (128, 128)), ("w2e", (33, 128)), ("w2d", (128, 256)),
               ("s2s", (128, 128)), ("s2e", (33, 128)),
               ("w3s", (128, 128)), ("w3e", (33, 128)), ("w3d", (128, 256)),
               ("s3s", (128, 128)), ("s3e", (33, 128)),
               ("wp_aug", (4, 128))]
    for nm, sh in wshapes:
        din(nm, sh, dt.bfloat16)
    hw1_d = din("hw1", (HIDDEN, HIDDEN), dt.float32)
    hb1_d = din("hb1", (1, HIDDEN), dt.float32)
    hw2_d = din("hw2", (HIDDEN, OUT_DIM), dt.float32)
    hb2_d = din("hb2", (1, OUT_DIM), dt.float32)

    out_d = nc.dram_tensor("out", [N_GRAPHS, OUT_DIM], dt.float32, kind="ExternalOutput")

    h_local = nc.dram_tensor("h_local", [NLP, 128], dt.bfloat16)
    h_full = nc.dram_tensor("h_full", [NFULL, 128], dt.bfloat16, addr_space="Shared")
    h2_local = nc.dram_tensor("h2_local", [NLP, 128], dt.bfloat16)
    h2_full = nc.dram_tensor("h2_full", [NFULL, 128], dt.bfloat16, addr_space="Shared")
    y_hbm = nc.dram_tensor("y_hbm", [128, NTILE * 256], dt.bfloat16)
    pool_in = nc.dram_tensor("pool_in", [N_GRAPHS, HIDDEN], dt.float32)
    pool_out = nc.dram_tensor("pool_out", [N_GRAPHS, HIDDEN], dt.float32,
                              addr_space="Shared")

    O = mybir.AluOpType
    A = mybir.ActivationFunctionType

    with tile.TileContext(nc, num_cores=N_CORES) as tc:
        with (
            tc.tile_pool(name="const", bufs=1) as cpool,
            tc.tile_pool(name="work", bufs=3) as pool,
            tc.tile_pool(name="gath", bufs=4) as gpool,
            tc.tile_pool(name="io", bufs=2) as ipool,
            tc.tile_pool(name="psfs", bufs=2, space="PSUM") as psum_fs,
            tc.tile_pool(name="psm", bufs=2, space="PSUM") as psum_m,
            tc.tile_pool(name="psag", bufs=1, space="PSUM") as psum_ag,
            tc.tile_pool(name="psy", bufs=1, space="PSUM") as psum_y,
        ):
            W = {}
            for nm, sh in wshapes:
                W[nm] = cpool.tile(list(sh), dt.bfloat16, tag=nm, name=f"w_{nm}")
                nc.sync.dma_start(out=W[nm][:], in_=D[nm][:])
            id_t = cpool.tile([128, 128], dt.bfloat16, tag="idt")
            idf_t = cpool.tile([128, 128], dt.float32, tag="idf")
            nc.sync.dma_start(out=id_t[:], in_=identb_d[:])
            nc.sync.dma_start(out=idf_t[:], in_=identf_d[:])
            gidx_t = cpool.tile([128, nslot // 16], dt.int16, tag="gidx")
            nc.sync.dma_start(out=gidx_t[:], in_=gsrc_d[:])
            xT_t = cpool.tile([4, NLP], dt.bfloat16, tag="xT")
            nc.sync.dma_start(out=xT_t[:], in_=xT_d[:])

            hT = cpool.tile([128, NLP], dt.bfloat16, tag="hT")

            pools = (pool, psum_fs, psum_m, psum_ag, gpool, ipool)

            # ---- conv1 Y (from xT) ----
            y1 = cpool.tile([128, NTILE * 8], dt.bfloat16, tag="y1")
            for t in range(NTILE):
                yp = psum_y.tile([128, 256], dt.float32, space="PSUM", tag="yf")
                nc.tensor.matmul(yp[:, 0:4], lhsT=xT_t[:, t * 128:(t + 1) * 128],
                                 rhs=W["w1d"][:], start=True, stop=True)
                nc.tensor.matmul(yp[:, 4:8], lhsT=xT_t[:, t * 128:(t + 1) * 128],
                                 rhs=W["s1d"][:], start=True, stop=True)
                nc.vector.tensor_copy(out=y1[:, t * 8:(t + 1) * 8], in_=yp[:, :8])

            # ---- conv1 ----
            agg1 = cpool.tile([128, NTILE * 4], dt.float32, tag="agg1")
            nc.vector.memset(agg1[:], 0.0)
            _conv_pass(nc, NODE_DIM, xpad_d, gidx_t, eaT_d, ohblk_d,
                       ("sbuf", y1), W["w1s"], W["w1e"], W["s1s"], W["s1e"],
                       id_t, schedule, batches, agg1, pools)

            # ---- lift: h = relu((x + agg1) @ Wp + bp), build hT ----
            xr = cpool.tile([128, NTILE * 4], dt.float32, tag="xr")
            nc.sync.dma_start(out=xr[:], in_=xrow_d[:])
            h0 = cpool.tile([128, NTILE * 4], dt.float32, tag="h0")
            nc.vector.tensor_add(out=h0[:], in0=xr[:], in1=agg1[:])
            for t in range(NTILE):
                h0t_ps = psum_y.tile([128, 256], dt.float32, space="PSUM", tag="yf")
                nc.tensor.transpose(h0t_ps[:4, :128], h0[:, t * 4:(t + 1) * 4], idf_t[:])
                h0aug = pool.tile([4, 128], dt.bfloat16, tag="h0aug")
                nc.vector.tensor_copy(out=h0aug[:, :], in_=h0t_ps[:4, :128])
                hl_ps = psum_y.tile([128, 256], dt.float32, space="PSUM", tag="yf")
                nc.tensor.matmul(hl_ps[:, :128], lhsT=h0aug[:], rhs=W["wp_aug"][:],
                                 start=True, stop=True)
                h_sb = pool.tile([128, 128], dt.bfloat16, tag="hsb")
                nc.scalar.activation(h_sb[:], hl_ps[:, :128], A.Relu)
                nc.sync.dma_start(out=h_local[t * 128:(t + 1) * 128, :], in_=h_sb[:])
                ht_ps = psum_m.tile([128, 128], dt.bfloat16, space="PSUM", tag="mt")
                nc.tensor.transpose(ht_ps[:], h_sb[:], id_t[:])
                nc.scalar.copy(out=hT[:, t * 128:(t + 1) * 128], in_=ht_ps[:])

            nc.gpsimd.collective_compute(
                "AllGather", O.bypass, replica_groups=[list(range(N_CORES))],
                ins=[h_local[:]], outs=[h_full[:]])

            def conv_hidden(wd_fs, wfs, wfe, wss, wse, table_full, aggH):
                # Y phase: y_hbm[:, t*256:...] = h_tile @ [Wfd | Wsd]
                for t in range(NTILE):
                    yp = psum_y.tile([128, 256], dt.float32, space="PSUM", tag="yf")
                    nc.tensor.matmul(yp[:, 0:256], lhsT=hT[:, t * 128:(t + 1) * 128],
                                     rhs=wd_fs[:], start=True, stop=True)
                    y_sb = pool.tile([128, 256], dt.bfloat16, tag="ysb")
                    nc.scalar.copy(out=y_sb[:], in_=yp[:])
                    nc.sync.dma_start(out=y_hbm[:, t * 256:(t + 1) * 256], in_=y_sb[:])
                nc.vector.memset(aggH[:], 0.0)
                _conv_pass(nc, HIDDEN, table_full, gidx_t, eaT_d, ohblk_d,
                           ("hbm", y_hbm), wfs, wfe, wss, wse,
                           id_t, schedule, batches, aggH, pools)

            # ---- conv2 ----
            aggH = cpool.tile([128, NTILE * 128], dt.float32, tag="aggH")
            conv_hidden(W["w2d"], W["w2s"], W["w2e"], W["s2s"], W["s2e"],
                        h_full, aggH)

            # update h2 = relu(h + aggH); write h2_local + hT
            for t in range(NTILE):
                hprev = ipool.tile([128, 128], dt.bfloat16, tag="hprev")
                nc.sync.dma_start(out=hprev[:], in_=h_local[t * 128:(t + 1) * 128, :])
                h2_sb = pool.tile([128, 128], dt.bfloat16, tag="h2sb")
                nc.vector.tensor_add(out=h2_sb[:], in0=aggH[:, t * 128:(t + 1) * 128],
                                     in1=hprev[:])
                nc.vector.tensor_scalar_max(out=h2_sb[:], in0=h2_sb[:], scalar1=0.0)
                nc.sync.dma_start(out=h2_local[t * 128:(t + 1) * 128, :], in_=h2_sb[:])
                ht_ps = psum_m.tile([128, 128], dt.bfloat16, space="PSUM", tag="mt")
                nc.tensor.transpose(ht_ps[:], h2_sb[:], id_t[:])
                nc.scalar.copy(out=hT[:, t * 128:(t + 1) * 128], in_=ht_ps[:])

            nc.gpsimd.collective_compute(
                "AllGather", O.bypass, replica_groups=[list(range(N_CORES))],
                ins=[h2_local[:]], outs=[h2_full[:]])

            # ---- conv3 ----
            agg3 = cpool.tile([128, NTILE * 128], dt.float32, tag="aggH")
            conv_hidden(W["w3d"], W["w3s"], W["w3e"], W["s3s"], W["s3e"],
                        h2_full, agg3)

            # ---- h3 = relu(h2 + agg3); pooling ----
            pl_full = psum_ag.tile([128, HIDDEN], dt.float32, space="PSUM", tag="ag")
            pl_ps = pl_full[:N_GRAPHS, :]
            for t in range(NTILE):
                hprev = ipool.tile([128, 128], dt.bfloat16, tag="hprev")
                nc.sync.dma_start(out=hprev[:], in_=h2_local[t * 128:(t + 1) * 128, :])
                indt = ipool.tile([128, N_GRAPHS], dt.float32, tag="indt")
                nc.sync.dma_start(out=indt[:],
                                  in_=ind_d[:, t * N_GRAPHS:(t + 1) * N_GRAPHS])
                indb = pool.tile([128, N_GRAPHS], dt.bfloat16, tag="indb")
                nc.vector.tensor_copy(out=indb[:], in_=indt[:])
                h3_sb = pool.tile([128, 128], dt.bfloat16, tag="h2sb")
                nc.vector.tensor_add(out=h3_sb[:], in0=agg3[:, t * 128:(t + 1) * 128],
                                     in1=hprev[:])
                nc.vector.tensor_scalar_max(out=h3_sb[:], in0=h3_sb[:], scalar1=0.0)
                nc.tensor.matmul(pl_ps, lhsT=indb[:], rhs=h3_sb[:],
                                 start=(t == 0), stop=(t == NTILE - 1))

            pl_sb = cpool.tile([N_GRAPHS, HIDDEN], dt.float32, tag="plsb")
            nc.vector.tensor_copy(out=pl_sb[:], in_=pl_ps)
            nc.sync.dma_start(out=pool_in[:], in_=pl_sb[:])
            nc.gpsimd.collective_compute(
                "AllReduce", O.add, replica_groups=[list(range(N_CORES))],
                ins=[pool_in[:]], outs=[pool_out[:]])

            # ---- head ----
            invc_t = cpool.tile([N_GRAPHS, 1], dt.float32, tag="invc")
            nc.sync.dma_start(out=invc_t[:], in_=invc_d[:])
            pooled = cpool.tile([N_GRAPHS, HIDDEN], dt.float32, tag="pooled")
            nc.sync.dma_start(out=pooled[:], in_=pool_out[:])
            nc.vector.tensor_scalar(out=pooled[:], in0=pooled[:],
                                    scalar1=invc_t[:, 0:1], scalar2=None, op0=O.mult)
            w1_t = cpool.tile([HIDDEN, HIDDEN], dt.float32, tag="w1")
            b1_t = cpool.tile([1, HIDDEN], dt.float32, tag="b1")
            w2_t = cpool.tile([HIDDEN, OUT_DIM], dt.float32, tag="w2")
            b2_t = cpool.tile([1, OUT_DIM], dt.float32, tag="b2")
            ones_g = cpool.tile([1, N_GRAPHS], dt.float32, tag="onesg")
            nc.vector.memset(ones_g[:], 1.0)
            for d_, s_ in [(w1_t, hw1_d), (b1_t, hb1_d), (w2_t, hw2_d), (b2_t, hb2_d)]:
                nc.sync.dma_start(out=d_[:], in_=s_[:])

            ptp = psum_y.tile([128, 256], dt.float32, space="PSUM", tag="yf")
            nc.tensor.transpose(ptp[:, :N_GRAPHS], pooled[:], idf_t[:N_GRAPHS, :N_GRAPHS])
            pooledT = cpool.tile([HIDDEN, N_GRAPHS], dt.float32, tag="pT")
            nc.vector.tensor_copy(out=pooledT[:], in_=ptp[:, :N_GRAPHS])
            hh_ps = psum_y.tile([128, 256], dt.float32, space="PSUM", tag="yf")
            nc.tensor.matmul(hh_ps[:N_GRAPHS, :128], lhsT=pooledT[:], rhs=w1_t[:],
                             start=True, stop=False)
            nc.tensor.matmul(hh_ps[:N_GRAPHS, :128], lhsT=ones_g[:], rhs=b1_t[:],
                             start=False, stop=True)
            hh = cpool.tile([N_GRAPHS, HIDDEN], dt.float32, tag="hh")
            nc.scalar.activation(hh[:], hh_ps[:N_GRAPHS, :128], A.Relu)
            htp = psum_y.tile([128, 256], dt.float32, space="PSUM", tag="yf")
            nc.tensor.transpose(htp[:, :N_GRAPHS], hh[:], idf_t[:N_GRAPHS, :N_GRAPHS])
            hhT = cpool.tile([HIDDEN, N_GRAPHS], dt.float32, tag="hhT")
            nc.vector.tensor_copy(out=hhT[:], in_=htp[:, :N_GRAPHS])
            out_ps = psum_y.tile([128, 256], dt.float32, space="PSUM", tag="yf")
            nc.tensor.matmul(out_ps[:N_GRAPHS, :OUT_DIM], lhsT=hhT[:], rhs=w2_t[:],
                             start=True, stop=False)
            nc.tensor.matmul(out_ps[:N_GRAPHS, :OUT_DIM], lhsT=ones_g[:], rhs=b2_t[:],
                             start=False, stop=True)
            out_sb = cpool.tile([N_GRAPHS, OUT_DIM], dt.float32, tag="osb")
            nc.vector.tensor_copy(out=out_sb[:], in_=out_ps[:N_GRAPHS, :OUT_DIM])
            nc.sync.dma_start(out=out_d[:], in_=out_sb[:])

    nc.compile()
    return nc


def kernel(**inputs) -> np.ndarray:
    in_maps, schedule, batches, nslot, maxgw = _prep(inputs)
    nc = build(schedule, batches, nslot, maxgw)
    res = run_bass_kernel_spmd(nc, in_maps, list(range(N_CORES)))
    return res.results[0]["out"].astype(np.float32)


# revision 18
# speedup vs baseline: 1.1409x; 1.1409x over previous
"""CGCNN (3x CGConv + pooled MLP head) on 8 TRN2 NeuronCores — v2.

Sharding: dst-range node sharding (core k owns nodes [k*12500,(k+1)*12500)).
Per core, the core's 12500 nodes are assigned to 98 tiles of 128 by a
balanced greedy packer so that per-(src-chunk, dst-tile) cell edge counts
are nearly equal across cells and cores (cuts slot padding to a few %).
Edges are ordered (src-chunk bucket, dst-tile); cell group counts are padded
to the max over cores (SPMD-identical program).

Key structure per conv:
  - src features: dma_gather (transpose mode) of 256B rows from the
    (AllGathered) node table, batched ~2048 idx/call to amortize fixed cost.
  - dst features: NO gather. Per tile, Y{f,s}_t = h_tile @ W{f,s}_dst is
    computed on the PE (from an SBUF-resident transposed local table), and
    the per-edge dst term is expanded with a host-precomputed one-hot
    matrix streamed from HBM: F_dst = Yf_t^T(lhsT) x oh2.
  - messages: m = sigmoid(F) * softplus(S) with native activations
    (2 scalar acts + 1 DVE mult per chunk).
  - aggregation: per 128-edge group, PE transpose of m + one-hot matmul
    (oh streamed from HBM next to oh2) accumulated per cell in PSUM.
h tables are bf16 in HBM, replicated by AllGather after conv1 and conv2.
Pooling via indicator matmul + AllReduce; small MLP head replicated fp32.
"""
import os
import numpy as np
import ml_dtypes

import concourse.bass as bass
import concourse.bacc as bacc
import concourse.tile as tile
from concourse import mybir
from concourse.bass_utils import run_bass_kernel_spmd

dt = mybir.dt
bf16 = ml_dtypes.bfloat16

N_NODES = 100000
NODE_DIM = 3
EDGE_DIM = 32
HIDDEN = 128
OUT_DIM = 3
N_GRAPHS = 64
N_CORES = 8
NL = N_NODES // N_CORES          # 12500
NTILE = (NL + 127) // 128        # 98
NLP = NTILE * 128                # 12544
NFULL = NLP * N_CORES            # 100352
NCHUNK = 4
CHUNK = NFULL // NCHUNK          # 25088 < 32768
GB = 1024                        # gather batch tile (slots)
GCALL = 512                      # max idx per dma_gather call
A_SP = 0.69219361
B_SP = 0.42078611


def _wrap16(idx):
    w = idx.reshape(-1, 16).T.astype(np.int16).copy()
    return np.tile(w, (8, 1))       # replicate across the 8 q7 cores


def _cell_targets():
    """Shared (cross-core) per-(tile,bucket) slot targets, multiples of 128.
    95 cells @512 + 3 cells @640 per bucket, large cells spread over tiles."""
    T = np.full((NTILE, NCHUNK), 512, np.int64)
    for b in range(NCHUNK):
        for j in range(3):
            T[(b * 3 + j) % NTILE, b] = 640
    return T


def _balance_tiles(deg, T):
    """deg: [NL, NCHUNK] per-node per-bucket in-degree. Pack nodes into 98
    tiles (<=128 nodes) aiming to keep per-(tile,bucket) sums under the
    shared targets T."""
    order = np.argsort(-deg.sum(1), kind="stable")
    sums = np.zeros((NTILE, NCHUNK), np.int64)
    cap = np.full(NTILE, 128, np.int64)
    tile_of = np.empty(NL, np.int64)
    Tf = T.astype(np.float64)
    for n in order:
        d = deg[n]
        if d.sum() == 0:
            t = int(np.argmax(cap))
        else:
            ns = sums + d
            over = np.maximum(ns - T, 0).sum(axis=1).astype(np.float64)
            frac = (ns / Tf).max(axis=1)
            score = over * 1e3 + frac
            score[cap <= 0] = np.inf
            t = int(np.argmin(score))
        tile_of[n] = t
        sums[t] += d
        cap[t] -= 1
    return tile_of, sums


def _prep(inputs):
    x = np.asarray(inputs["x"], np.float32)
    ei = np.asarray(inputs["edge_index"])
    ea = np.asarray(inputs["edge_attr"], np.float32)
    batch = np.asarray(inputs["batch"]).astype(np.int64)
    src, dst_g = ei[0].astype(np.int64), ei[1].astype(np.int64)

    owner = dst_g // NL
    b_edge = (src // NL) // 2          # == pad_id // CHUNK, permutation-free

    # --- balanced tile assignment per core ---
    tile_of_g = np.empty(N_NODES, np.int64)
    pos_g = np.empty(N_NODES, np.int64)     # local padded id within owner
    cnts = np.zeros((N_CORES, NCHUNK, NTILE), np.int64)
    per_core_sel = []
    Tgt = _cell_targets()
    for k in range(N_CORES):
        sel = np.nonzero(owner == k)[0]
        d_loc = dst_g[sel] - k * NL
        deg = np.zeros((NL, NCHUNK), np.int64)
        np.add.at(deg, (d_loc, b_edge[sel]), 1)
        tile_of, sums = _balance_tiles(deg, Tgt)
        cnts[k] = sums.T                     # [NCHUNK, NTILE]
        # positions within tile: stable by node id
        pos_in = np.zeros(NL, np.int64)
        for t in range(NTILE):
            nodes = np.nonzero(tile_of == t)[0]
            pos_in[nodes] = np.arange(len(nodes))
        tile_of_g[k * NL:(k + 1) * NL] = tile_of
        pos_g[k * NL:(k + 1) * NL] = tile_of * 128 + pos_in
        per_core_sel.append((sel, d_loc))

    pid_g = (np.arange(N_NODES) // NL) * NLP + pos_g   # padded global row

    ngroups = np.ceil(cnts / 128.0).astype(np.int64).max(axis=0)  # [NCHUNK, NTILE]
    schedule = []        # (b, t, off, gw)
    off = 0
    for b in range(NCHUNK):
        for t in range(NTILE):
            g = int(ngroups[b, t])
            if g == 0:
                continue
            schedule.append((b, t, off, g * 128))
            off += g * 128
    nslot = off
    maxgw = max(s[3] for s in schedule)
    assert maxgw <= 768, maxgw

    # gather batches: cell-aligned, <= GB slots, single bucket
    batches = []         # (off, n, bkt)
    cur_off, cur_n, cur_b = None, 0, None
    for (b, t, o, gw) in schedule:
        if cur_off is not None and (b != cur_b or cur_n + gw > GB):
            batches.append((cur_off, cur_n, cur_b))
            cur_off = None
        if cur_off is None:
            cur_off, cur_n, cur_b = o, 0, b
        cur_n += gw
    if cur_off is not None:
        batches.append((cur_off, cur_n, cur_b))

    # --- weights ---
    Wf1 = np.asarray(inputs["Wf1"], np.float32); bf1 = np.asarray(inputs["bf1"], np.float32)
    Ws1 = np.asarray(inputs["Ws1"], np.float32); bs1 = np.asarray(inputs["bs1"], np.float32)
    Wp = np.asarray(inputs["Wp"], np.float32); bp = np.asarray(inputs["bp"], np.float32)
    P = {nm: np.asarray(inputs[nm], np.float32) for nm in
         ["Wf2", "bf2", "Ws2", "bs2", "Wf3", "bf3", "Ws3", "bs3", "W1", "b1", "W2", "b2"]}

    def cw(W, b):
        ws = W[HIDDEN:2 * HIDDEN]                     # src part  [128,128]
        we = np.zeros((33, HIDDEN), np.float32)
        we[:32] = W[2 * HIDDEN:]
        we[32] = b
        wd = W[:HIDDEN]                               # dst part (rhs for Y)
        return ws.astype(bf16), we.astype(bf16), wd.astype(bf16)

    w2s, w2e, w2da = cw(P["Wf2"], P["bf2"]); s2s, s2e, s2da = cw(P["Ws2"], P["bs2"])
    w3s, w3e, w3da = cw(P["Wf3"], P["bf3"]); s3s, s3e, s3da = cw(P["Ws3"], P["bs3"])
    w2d = np.concatenate([w2da, s2da], axis=1)        # [128, 256]
    w3d = np.concatenate([w3da, s3da], axis=1)

    def c1w(W, b):
        ws = np.zeros((128, 4), np.float32); ws[0:3, :3] = W[3:6]
        we = np.zeros((33, 4), np.float32); we[:32, :3] = W[6:]; we[32, :3] = b
        wd = np.zeros((4, 4), np.float32); wd[0:3, :3] = W[0:3]
        return ws.astype(bf16), we.astype(bf16), wd.astype(bf16)

    w1s, w1e, w1d = c1w(Wf1, bf1); s1s, s1e, s1d = c1w(Ws1, bs1)
    wp_aug = np.zeros((4, HIDDEN), np.float32); wp_aug[:3] = Wp; wp_aug[3] = bp

    gcnts = np.bincount(batch, minlength=N_GRAPHS).astype(np.float32)
    inv_cnt = (1.0 / np.maximum(gcnts, 1.0)).reshape(N_GRAPHS, 1)

    identb = np.eye(128, dtype=np.float32).astype(bf16)
    identf = np.eye(128, dtype=np.float32)

    xpad = np.zeros((NFULL, 128), bf16)
    xpad[pid_g, :3] = x.astype(bf16)

    in_maps = []
    for k in range(N_CORES):
        sel, d_loc = per_core_sel[k]
        dslot = pos_g[k * NL:(k + 1) * NL][d_loc]          # padded local id of dst
        d_tile = dslot // 128
        d_in_tile = dslot % 128
        eorder = np.lexsort((d_in_tile, d_tile, b_edge[sel]))
        es, dt_s, dit_s, bb = (sel[eorder], d_tile[eorder],
                               d_in_tile[eorder], b_edge[sel][eorder])

        gsrc = np.zeros(nslot, np.int16)
        eaT = np.zeros((33, nslot), np.float32)
        ohblk = np.zeros((128, 2 * nslot), bf16)
        ptr = 0
        n_e = len(es)
        for (b, t, o, gw) in schedule:
            p2 = ptr
            while p2 < n_e and bb[p2] == b and dt_s[p2] == t:
                p2 += 1
            cnt = p2 - ptr
            assert cnt <= gw
            gsrc[o:o + cnt] = (pid_g[src[es[ptr:p2]]] % CHUNK).astype(np.int16)
            eaT[:32, o:o + cnt] = ea[es[ptr:p2]].T
            eaT[32, o:o + cnt] = 1.0
            dd = dit_s[ptr:p2]
            # oh2 [node_slot, edge]
            ohblk[dd, 2 * o + np.arange(cnt)] = 1.0
            # oh per group [edge_in_group, node_slot]; 0.5 folds sigmoid's /2
            jj = np.arange(cnt)
            ohblk[jj % 128, 2 * o + gw + (jj // 128) * 128 + dd] = 0.5
            ptr = p2
        assert ptr == n_e

        xT_sb = np.zeros((4, NLP), np.float32)
        xl = x[k * NL:(k + 1) * NL]
        pidl = pos_g[k * NL:(k + 1) * NL]
        xT_sb[:3, pidl] = xl.T

        xrow = np.zeros((128, NTILE * 4), np.float32)
        xrow[pidl % 128, (pidl // 128) * 4 + 0] = xl[:, 0]
        xrow[pidl % 128, (pidl // 128) * 4 + 1] = xl[:, 1]
        xrow[pidl % 128, (pidl // 128) * 4 + 2] = xl[:, 2]
        xrow[:, 3::4] = 1.0

        ind = np.zeros((NLP, N_GRAPHS), np.float32)
        ind[pidl, batch[k * NL:(k + 1) * NL]] = 1.0

        in_maps.append(dict(
            xpad=xpad,
            gsrc=_wrap16(gsrc),
            eaT=eaT.astype(bf16),
            ohblk=ohblk,
            xT_sb=xT_sb.astype(bf16),
            xrow=xrow,
            identb=identb, identf=identf,
            ind=ind.reshape(NTILE, 128, N_GRAPHS).transpose(1, 0, 2)
                  .reshape(128, NTILE * N_GRAPHS).copy(),
            inv_cnt=inv_cnt,
            w1s=w1s, w1e=w1e, w1d=w1d, s1s=s1s, s1e=s1e, s1d=s1d,
            w2s=w2s, w2e=w2e, w2d=w2d, s2s=s2s, s2e=s2e,
            w3s=w3s, w3e=w3e, w3d=w3d, s3s=s3s, s3e=s3e,
            wp_aug=wp_aug.astype(bf16),
            hw1=P["W1"], hb1=P["b1"].reshape(1, HIDDEN).copy(),
            hw2=P["W2"], hb2=P["b2"].reshape(1, OUT_DIM).copy(),
        ))
    return in_maps, schedule, batches, nslot, maxgw


def _conv_pass(nc, cdim, table_d, gidx_t, eaT_d, ohblk_d, y_src, wfs, wfe,
               wss, wse, id_t, schedule, batches, agg, pools):
    """y_src: ('sbuf', tile) with [128, NTILE*8] for conv1 (f cols 0:4, s 4:8)
    or ('hbm', dram) with [128, NTILE*256] for conv2/3 (f 0:128, s 128:256)."""
    pool, psum_fs, psum_m, psum_ag, gpool, ipool = pools
    cp = 4 if cdim == 3 else 128
    acols = 4 if cdim == 3 else 128
    A = mybir.ActivationFunctionType

    # map cell -> batch
    cell_batch = {}
    for bi, (boff, bn, bkt) in enumerate(batches):
        for ci, (b, t, o, gw) in enumerate(schedule):
            if boff <= o < boff + bn:
                cell_batch[ci] = bi

    def gather(bi):
        boff, bn, bkt = batches[bi]
        hb = gpool.tile([128, GB], dt.bfloat16, tag="hsrc")
        for j in range(0, bn, GCALL):
            n = min(GCALL, bn - j)
            nc.gpsimd.dma_gather(
                out_ap=hb[:, j:j + n].rearrange("p (g e) -> p g e", g=1),
                in_ap=table_d[bkt * CHUNK:(bkt + 1) * CHUNK, :],
                idxs_ap=gidx_t[:, (boff + j) // 16:(boff + j + n) // 16],
                num_idxs=n, num_idxs_reg=n, elem_size=128, transpose=True)
        return hb

    pending = {}
    pending[0] = gather(0)
    if len(batches) > 1:
        pending[1] = gather(1)
    cur_bi = 0
    hb = pending.pop(0)
    for ci, (b, t, o, gw) in enumerate(schedule):
        bi = cell_batch[ci]
        if bi != cur_bi:
            hb = pending.pop(bi) if bi in pending else gather(bi)
            cur_bi = bi
        for pf in (bi + 1, bi + 2, bi + 3):
            if pf < len(batches) and pf not in pending and pf > bi:
                if all(p <= bi or p in pending for p in range(bi + 1, pf)):
                    if pf - bi <= 3 and pf not in pending:
                        pending[pf] = gather(pf)

        boff = batches[bi][0]
        oh_t = ipool.tile([128, 2 * 768], dt.bfloat16, tag="oh")
        nc.sync.dma_start(out=oh_t[:, :2 * gw], in_=ohblk_d[:, 2 * o:2 * o + 2 * gw])
        ea_t = ipool.tile([33, 768], dt.bfloat16, tag="ea")
        nc.sync.dma_start(out=ea_t[:, :gw], in_=eaT_d[:, o:o + gw])
        if y_src[0] == "hbm":
            y_t = ipool.tile([128, 256], dt.bfloat16, tag="yy")
            nc.sync.dma_start(out=y_t[:], in_=y_src[1][:, t * 256:(t + 1) * 256])
            yf, ys = y_t[:, 0:cdim], y_t[:, 128:128 + cdim]
        else:
            y_t = y_src[1]
            yf, ys = y_t[:, t * 8:t * 8 + 4], y_t[:, t * 8 + 4:t * 8 + 8]

        ng = gw // 128
        ag = psum_ag.tile([128, acols], dt.float32, space="PSUM", tag="ag",
                          name=f"ag{ci}")
        gg = 0
        for c0 in range(0, ng, 4):
            cn = min(4, ng - c0)
            cw_ = cn * 128
            co = c0 * 128
            f_ps = psum_fs.tile([128, 512], dt.float32, space="PSUM", tag="f")
            s_ps = psum_fs.tile([128, 512], dt.float32, space="PSUM", tag="s")
            hs = hb[:, o - boff + co:o - boff + co + cw_]
            eb = ea_t[:33, co:co + cw_]
            o2 = oh_t[:, co:co + cw_]
            nc.tensor.matmul(f_ps[:cp, :cw_], lhsT=wfs[:], rhs=hs, start=True, stop=False)
            nc.tensor.matmul(f_ps[:cp, :cw_], lhsT=wfe[:], rhs=eb, start=False, stop=False)
            nc.tensor.matmul(f_ps[:cp, :cw_], lhsT=yf, rhs=o2, start=False, stop=True)
            nc.tensor.matmul(s_ps[:cp, :cw_], lhsT=wss[:], rhs=hs, start=True, stop=False)
            nc.tensor.matmul(s_ps[:cp, :cw_], lhsT=wse[:], rhs=eb, start=False, stop=False)
            nc.tensor.matmul(s_ps[:cp, :cw_], lhsT=ys, rhs=o2, start=False, stop=True)

            # m' = (tanh(F/2)+1) * (silu(S) + A - A*tanh^2(B*S));  the /2 of
            # sigmoid is folded into the one-hot aggregation values (0.5)
            O = mybir.AluOpType
            tf = pool.tile([128, 512], dt.bfloat16, tag="tf")
            ss = pool.tile([128, 512], dt.bfloat16, tag="ss")
            ts = pool.tile([128, 512], dt.bfloat16, tag="ts")
            nc.scalar.activation(tf[:cp, :cw_], f_ps[:cp, :cw_], A.Tanh, scale=0.5)
            nc.scalar.activation(ss[:cp, :cw_], s_ps[:cp, :cw_], A.Silu)
            nc.scalar.activation(ts[:cp, :cw_], s_ps[:cp, :cw_], A.Tanh, scale=B_SP)
            t1 = pool.tile([128, 512], dt.bfloat16, tag="t1")
            nc.vector.scalar_tensor_tensor(out=t1[:cp, :cw_], in0=ts[:cp, :cw_],
                                           scalar=-A_SP, in1=ts[:cp, :cw_],
                                           op0=O.mult, op1=O.mult)
            t2 = pool.tile([128, 512], dt.bfloat16, tag="t2")
            nc.vector.scalar_tensor_tensor(out=t2[:cp, :cw_], in0=t1[:cp, :cw_],
                                           scalar=A_SP, in1=ss[:cp, :cw_],
                                           op0=O.add, op1=O.add)
            m_bf = pool.tile([128, 512], dt.bfloat16, tag="mbf")
            nc.vector.scalar_tensor_tensor(out=m_bf[:cp, :cw_], in0=tf[:cp, :cw_],
                                           scalar=1.0, in1=t2[:cp, :cw_],
                                           op0=O.add, op1=O.mult)

            for g in range(cn):
                m_ps = psum_m.tile([128, 128], dt.bfloat16, space="PSUM", tag="mt")
                nc.tensor.transpose(m_ps[:, :cdim], m_bf[:cdim, g * 128:(g + 1) * 128],
                                    id_t[:cdim, :cdim])
                m_sb = pool.tile([128, 128], dt.bfloat16, tag="msb")
                if (gg % 2) == 0:
                    nc.vector.tensor_copy(out=m_sb[:, :cdim], in_=m_ps[:, :cdim])
                else:
                    nc.scalar.copy(out=m_sb[:, :cdim], in_=m_ps[:, :cdim])
                ohg = oh_t[:, gw + (gg * 128):gw + (gg + 1) * 128]
                nc.tensor.matmul(ag[:, :cdim], lhsT=ohg, rhs=m_sb[:, :cdim],
                                 start=(gg == 0), stop=(gg == ng - 1))
                gg += 1
        nc.vector.tensor_add(out=agg[:, t * acols:t * acols + cdim],
                             in0=agg[:, t * acols:t * acols + cdim],
                             in1=ag[:, :cdim])


def build(schedule, batches, nslot, maxgw):
    nc = bacc.Bacc("TRN2", target_bir_lowering=False, debug=False, num_devices=N_CORES)
    D = {}

    def din(name, shape, dtype):
        D[name] = nc.dram_tensor(name, list(shape), dtype, kind="ExternalInput")
        return D[name]

    xpad_d = din("xpad", (NFULL, 128), dt.bfloat16)
    gsrc_d = din("gsrc", (128, nslot // 16), dt.int16)
    eaT_d = din("eaT", (33, nslot), dt.bfloat16)
    ohblk_d = din("ohblk", (128, 2 * nslot), dt.bfloat16)
    xT_d = din("xT_sb", (4, NLP), dt.bfloat16)
    xrow_d = din("xrow", (128, NTILE * 4), dt.float32)
    identb_d = din("identb", (128, 128), dt.bfloat16)
    identf_d = din("identf", (128, 128), dt.float32)
    ind_d = din("ind", (128, NTILE * N_GRAPHS), dt.float32)
    invc_d = din("inv_cnt", (N_GRAPHS, 1), dt.float32)
    wshapes = [("w1s", (128, 4)), ("w1e", (33, 4)), ("w1d", (4, 4)),
               ("s1s", (128, 4)), ("s1e", (33, 4)), ("s1d", (4, 4)),
               ("w2s", (128, 128)), ("w2e", (33, 128)), ("w2d", (128, 256)),
               ("s2s", (128, 128)), ("s2e", (33, 128)),
               ("w3s", (128, 128)), ("w3e", (33, 128)), ("w3d", (128, 256)),
               ("s3s", (128, 128)), ("s3e", (33, 128)),
               ("wp_aug", (4, 128))]
    for nm, sh in wshapes:
        din(nm, sh, dt.bfloat16)
    hw1_d = din("hw1", (HIDDEN, HIDDEN), dt.float32)
    hb1_d = din("hb1", (1, HIDDEN), dt.float32)
    hw2_d = din("hw2", (HIDDEN, OUT_DIM), dt.float32)
    hb2_d = din("hb2", (1, OUT_DIM), dt.float32)

    out_d = nc.dram_tensor("out", [N_GRAPHS, OUT_DIM], dt.float32, kind="ExternalOutput")

    h_local = nc.dram_tensor("h_local", [NLP, 128], dt.bfloat16)
    h_full = nc.dram_tensor("h_full", [NFULL, 128], dt.bfloat16, addr_space="Shared")
    h2_local = nc.dram_tensor("h2_local", [NLP, 128], dt.bfloat16)
    h2_full = nc.dram_tensor("h2_full", [NFULL, 128], dt.bfloat16, addr_space="Shared")
    y_hbm = nc.dram_tensor("y_hbm", [128, NTILE * 256], dt.bfloat16)
    pool_in = nc.dram_tensor("pool_in", [N_GRAPHS, HIDDEN], dt.float32)
    pool_out = nc.dram_tensor("pool_out", [N_GRAPHS, HIDDEN], dt.float32,
                              addr_space="Shared")

    O = mybir.AluOpType
    A = mybir.ActivationFunctionType

    with tile.TileContext(nc, num_cores=N_CORES) as tc:
        with (
            tc.tile_pool(name="const", bufs=1) as cpool,
            tc.tile_pool(name="work", bufs=3) as pool,
            tc.tile_pool(name="gath", bufs=6) as gpool,
            tc.tile_pool(name="io", bufs=2) as ipool,
            tc.tile_pool(name="psfs", bufs=2, space="PSUM") as psum_fs,
            tc.tile_pool(name="psm", bufs=2, space="PSUM") as psum_m,
            tc.tile_pool(name="psag", bufs=1, space="PSUM") as psum_ag,
            tc.tile_pool(name="psy", bufs=1, space="PSUM") as psum_y,
        ):
            W = {}
            for nm, sh in wshapes:
                W[nm] = cpool.tile(list(sh), dt.bfloat16, tag=nm, name=f"w_{nm}")
                nc.sync.dma_start(out=W[nm][:], in_=D[nm][:])
            id_t = cpool.tile([128, 128], dt.bfloat16, tag="idt")
            idf_t = cpool.tile([128, 128], dt.float32, tag="idf")
            nc.sync.dma_start(out=id_t[:], in_=identb_d[:])
            nc.sync.dma_start(out=idf_t[:], in_=identf_d[:])
            gidx_t = cpool.tile([128, nslot // 16], dt.int16, tag="gidx")
            nc.sync.dma_start(out=gidx_t[:], in_=gsrc_d[:])
            xT_t = cpool.tile([4, NLP], dt.bfloat16, tag="xT")
            nc.sync.dma_start(out=xT_t[:], in_=xT_d[:])

            hT = cpool.tile([128, NLP], dt.bfloat16, tag="hT")

            pools = (pool, psum_fs, psum_m, psum_ag, gpool, ipool)

            # ---- conv1 Y (from xT) ----
            y1 = cpool.tile([128, NTILE * 8], dt.bfloat16, tag="y1")
            for t in range(NTILE):
                yp = psum_y.tile([128, 256], dt.float32, space="PSUM", tag="yf")
                nc.tensor.matmul(yp[:, 0:4], lhsT=xT_t[:, t * 128:(t + 1) * 128],
                                 rhs=W["w1d"][:], start=True, stop=True)
                nc.tensor.matmul(yp[:, 4:8], lhsT=xT_t[:, t * 128:(t + 1) * 128],
                                 rhs=W["s1d"][:], start=True, stop=True)
                nc.vector.tensor_copy(out=y1[:, t * 8:(t + 1) * 8], in_=yp[:, :8])

            # ---- conv1 ----
            agg1 = cpool.tile([128, NTILE * 4], dt.float32, tag="agg1")
            nc.vector.memset(agg1[:], 0.0)
            _conv_pass(nc, NODE_DIM, xpad_d, gidx_t, eaT_d, ohblk_d,
                       ("sbuf", y1), W["w1s"], W["w1e"], W["s1s"], W["s1e"],
                       id_t, schedule, batches, agg1, pools)

            # ---- lift: h = relu((x + agg1) @ Wp + bp), build hT ----
            xr = cpool.tile([128, NTILE * 4], dt.float32, tag="xr")
            nc.sync.dma_start(out=xr[:], in_=xrow_d[:])
            h0 = cpool.tile([128, NTILE * 4], dt.float32, tag="h0")
            nc.vector.tensor_add(out=h0[:], in0=xr[:], in1=agg1[:])
            for t in range(NTILE):
                h0t_ps = psum_y.tile([128, 256], dt.float32, space="PSUM", tag="yf")
                nc.tensor.transpose(h0t_ps[:4, :128], h0[:, t * 4:(t + 1) * 4], idf_t[:])
                h0aug = pool.tile([4, 128], dt.bfloat16, tag="h0aug")
                nc.vector.tensor_copy(out=h0aug[:, :], in_=h0t_ps[:4, :128])
                hl_ps = psum_y.tile([128, 256], dt.float32, space="PSUM", tag="yf")
                nc.tensor.matmul(hl_ps[:, :128], lhsT=h0aug[:], rhs=W["wp_aug"][:],
                                 start=True, stop=True)
                h_sb = pool.tile([128, 128], dt.bfloat16, tag="hsb")
                nc.scalar.activation(h_sb[:], hl_ps[:, :128], A.Relu)
                nc.sync.dma_start(out=h_local[t * 128:(t + 1) * 128, :], in_=h_sb[:])
                ht_ps = psum_m.tile([128, 128], dt.bfloat16, space="PSUM", tag="mt")
                nc.tensor.transpose(ht_ps[:], h_sb[:], id_t[:])
                nc.scalar.copy(out=hT[:, t * 128:(t + 1) * 128], in_=ht_ps[:])

            nc.gpsimd.collective_compute(
                "AllGather", O.bypass, replica_groups=[list(range(N_CORES))],
                ins=[h_local[:]], outs=[h_full[:]])

            def conv_hidden(wd_fs, wfs, wfe, wss, wse, table_full, aggH):
                # Y phase: y_hbm[:, t*256:...] = h_tile @ [Wfd | Wsd]
                for t in range(NTILE):
                    yp = psum_y.tile([128, 256], dt.float32, space="PSUM", tag="yf")
                    nc.tensor.matmul(yp[:, 0:256], lhsT=hT[:, t * 128:(t + 1) * 128],
                                     rhs=wd_fs[:], start=True, stop=True)
                    y_sb = pool.tile([128, 256], dt.bfloat16, tag="ysb")
                    nc.scalar.copy(out=y_sb[:], in_=yp[:])
                    nc.sync.dma_start(out=y_hbm[:, t * 256:(t + 1) * 256], in_=y_sb[:])
                nc.vector.memset(aggH[:], 0.0)
                _conv_pass(nc, HIDDEN, table_full, gidx_t, eaT_d, ohblk_d,
                           ("hbm", y_hbm), wfs, wfe, wss, wse,
                           id_t, schedule, batches, aggH, pools)

            # ---- conv2 ----
            aggH = cpool.tile([128, NTILE * 128], dt.float32, tag="aggH")
            conv_hidden(W["w2d"], W["w2s"], W["w2e"], W["s2s"], W["s2e"],
                        h_full, aggH)

            # update h2 = relu(h + aggH); write h2_local + hT
            for t in range(NTILE):
                hprev = ipool.tile([128, 128], dt.bfloat16, tag="hprev")
                nc.sync.dma_start(out=hprev[:], in_=h_local[t * 128:(t + 1) * 128, :])
                h2_sb = pool.tile([128, 128], dt.bfloat16, tag="h2sb")
                nc.vector.tensor_add(out=h2_sb[:], in0=aggH[:, t * 128:(t + 1) * 128],
                                     in1=hprev[:])
                nc.vector.tensor_scalar_max(out=h2_sb[:], in0=h2_sb[:], scalar1=0.0)
                nc.sync.dma_start(out=h2_local[t * 128:(t + 1) * 128, :], in_=h2_sb[:])
                ht_ps = psum_m.tile([128, 128], dt.bfloat16, space="PSUM", tag="mt")
                nc.tensor.transpose(ht_ps[:], h2_sb[:], id_t[:])
                nc.scalar.copy(out=hT[:, t * 128:(t + 1) * 128], in_=ht_ps[:])

            nc.gpsimd.collective_compute(
                "AllGather", O.bypass, replica_groups=[list(range(N_CORES))],
                ins=[h2_local[:]], outs=[h2_full[:]])

            # ---- conv3 ----
            agg3 = cpool.tile([128, NTILE * 128], dt.float32, tag="aggH")
            conv_hidden(W["w3d"], W["w3s"], W["w3e"], W["s3s"], W["s3e"],
                        h2_full, agg3)

            # ---- h3 = relu(h2 + agg3); pooling ----
            pl_full = psum_ag.tile([128, HIDDEN], dt.float32, space="PSUM", tag="ag")
            pl_ps = pl_full[:N_GRAPHS, :]
            for t in range(NTILE):
                hprev = ipool.tile([128, 128], dt.bfloat16, tag="hprev")
                nc.sync.dma_start(out=hprev[:], in_=h2_local[t * 128:(t + 1) * 128, :])
                indt = ipool.tile([128, N_GRAPHS], dt.float32, tag="indt")
                nc.sync.dma_start(out=indt[:],
                                  in_=ind_d[:, t * N_GRAPHS:(t + 1) * N_GRAPHS])
                indb = pool.tile([128, N_GRAPHS], dt.bfloat16, tag="indb")
                nc.vector.tensor_copy(out=indb[:], in_=indt[:])
                h3_sb = pool.tile([128, 128], dt.bfloat16, tag="h2sb")
                nc.vector.tensor_add(out=h3_sb[:], in0=agg3[:, t * 128:(t + 1) * 128],
                                     in1=hprev[:])
                nc.vector.tensor_scalar_max(out=h3_sb[:], in0=h3_sb[:], scalar1=0.0)
                nc.tensor.matmul(pl_ps, lhsT=indb[:], rhs=h3_sb[:],
                                 start=(t == 0), stop=(t == NTILE - 1))

            pl_sb = cpool.tile([N_GRAPHS, HIDDEN], dt.float32, tag="plsb")
            nc.vector.tensor_copy(out=pl_sb[:], in_=pl_ps)
            nc.sync.dma_start(out=pool_in[:], in_=pl_sb[:])
            nc.gpsimd.collective_compute(
                "AllReduce", O.add, replica_groups=[list(range(N_CORES))],
                ins=[pool_in[:]], outs=[pool_out[:]])

            # ---- head ----
            invc_t = cpool.tile([N_GRAPHS, 1], dt.float32, tag="invc")
            nc.sync.dma_start(out=invc_t[:], in_=invc_d[:])
            pooled = cpool.tile([N_GRAPHS, HIDDEN], dt.float32, tag="pooled")
            nc.sync.dma_start(out=pooled[:], in_=pool_out[:])
            nc.vector.tensor_scalar(out=pooled[:], in0=pooled[:],
                                    scalar1=invc_t[:, 0:1], scalar2=None, op0=O.mult)
            w1_t = cpool.tile([HIDDEN, HIDDEN], dt.float32, tag="w1")
            b1_t = cpool.tile([1, HIDDEN], dt.float32, tag="b1")
            w2_t = cpool.tile([HIDDEN, OUT_DIM], dt.float32, tag="w2")
            b2_t = cpool.tile([1, OUT_DIM], dt.float32, tag="b2")
            ones_g = cpool.tile([1, N_GRAPHS], dt.float32, tag="onesg")
            nc.vector.memset(ones_g[:], 1.0)
            for d_, s_ in [(w1_t, hw1_d), (b1_t, hb1_d), (w2_t, hw2_d), (b2_t, hb2_d)]:
                nc.sync.dma_start(out=d_[:], in_=s_[:])

            ptp = psum_y.tile([128, 256], dt.float32, space="PSUM", tag="yf")
            nc.tensor.transpose(ptp[:, :N_GRAPHS], pooled[:], idf_t[:N_GRAPHS, :N_GRAPHS])
            pooledT = cpool.tile([HIDDEN, N_GRAPHS], dt.float32, tag="pT")
            nc.vector.tensor_copy(out=pooledT[:], in_=ptp[:, :N_GRAPHS])
            hh_ps = psum_y.tile([128, 256], dt.float32, space="PSUM", tag="yf")
            nc.tensor.matmul(hh_ps[:N_GRAPHS, :128], lhsT=pooledT[:], rhs=w1_t[:],
                             start=True, stop=False)
            nc.tensor.matmul(hh_ps[:N_GRAPHS, :128], lhsT=ones_g[:], rhs=b1_t[:],
                             start=False, stop=True)
            hh = cpool.tile([N_GRAPHS, HIDDEN], dt.float32, tag="hh")
            nc.scalar.activation(hh[:], hh_ps[:N_GRAPHS, :128], A.Relu)
            htp = psum_y.tile([128, 256], dt.float32, space="PSUM", tag="yf")
            nc.tensor.transpose(htp[:, :N_GRAPHS], hh[:], idf_t[:N_GRAPHS, :N_GRAPHS])
            hhT = cpool.tile([HIDDEN, N_GRAPHS], dt.float32, tag="hhT")
            nc.vector.tensor_copy(out=hhT[:], in_=htp[:, :N_GRAPHS])
            out_ps = psum_y.tile([128, 256], dt.float32, space="PSUM", tag="yf")
            nc.tensor.matmul(out_ps[:N_GRAPHS, :OUT_DIM], lhsT=hhT[:], rhs=w2_t[:],
                             start=True, stop=False)
            nc.tensor.matmul(out_ps[:N_GRAPHS, :OUT_DIM], lhsT=ones_g[:], rhs=b2_t[:],
                             start=False, stop=True)
            out_sb = cpool.tile([N_GRAPHS, OUT_DIM], dt.float32, tag="osb")
            nc.vector.tensor_copy(out=out_sb[:], in_=out_ps[:N_GRAPHS, :OUT_DIM])
            nc.sync.dma_start(out=out_d[:], in_=out_sb[:])

    nc.compile()
    return nc


def kernel(**inputs) -> np.ndarray:
    in_maps, schedule, batches, nslot, maxgw = _prep(inputs)
    nc = build(schedule, batches, nslot, maxgw)
    res = run_bass_kernel_spmd(nc, in_maps, list(range(N_CORES)))
    return res.results[0]["out"].astype(np.float32)


# revision 19
# speedup vs baseline: 1.1427x; 1.0016x over previous
"""CGCNN (3x CGConv + pooled MLP head) on 8 TRN2 NeuronCores — v2.

Sharding: dst-range node sharding (core k owns nodes [k*12500,(k+1)*12500)).
Per core, the core's 12500 nodes are assigned to 98 tiles of 128 by a
balanced greedy packer so that per-(src-chunk, dst-tile) cell edge counts
are nearly equal across cells and cores (cuts slot padding to a few %).
Edges are ordered (src-chunk bucket, dst-tile); cell group counts are padded
to the max over cores (SPMD-identical program).

Key structure per conv:
  - src features: dma_gather (transpose mode) of 256B rows from the
    (AllGathered) node table, batched ~2048 idx/call to amortize fixed cost.
  - dst features: NO gather. Per tile, Y{f,s}_t = h_tile @ W{f,s}_dst is
    computed on the PE (from an SBUF-resident transposed local table), and
    the per-edge dst term is expanded with a host-precomputed one-hot
    matrix streamed from HBM: F_dst = Yf_t^T(lhsT) x oh2.
  - messages: m = sigmoid(F) * softplus(S) with native activations
    (2 scalar acts + 1 DVE mult per chunk).
  - aggregation: per 128-edge group, PE transpose of m + one-hot matmul
    (oh streamed from HBM next to oh2) accumulated per cell in PSUM.
h tables are bf16 in HBM, replicated by AllGather after conv1 and conv2.
Pooling via indicator matmul + AllReduce; small MLP head replicated fp32.
"""
import os
import numpy as np
import ml_dtypes

import concourse.bass as bass
import concourse.bacc as bacc
import concourse.tile as tile
from concourse import mybir
from concourse.bass_utils import run_bass_kernel_spmd

dt = mybir.dt
bf16 = ml_dtypes.bfloat16

N_NODES = 100000
NODE_DIM = 3
EDGE_DIM = 32
HIDDEN = 128
OUT_DIM = 3
N_GRAPHS = 64
N_CORES = 8
NL = N_NODES // N_CORES          # 12500
NTILE = (NL + 127) // 128        # 98
NLP = NTILE * 128                # 12544
NFULL = NLP * N_CORES            # 100352
NCHUNK = 4
CHUNK = NFULL // NCHUNK          # 25088 < 32768
GB = 1024                        # gather batch tile (slots)
GCALL = 512                      # max idx per dma_gather call
A_SP = 0.69219361
B_SP = 0.42078611


def _wrap16(idx):
    w = idx.reshape(-1, 16).T.astype(np.int16).copy()
    return np.tile(w, (8, 1))       # replicate across the 8 q7 cores


def _cell_targets():
    """Shared (cross-core) per-(tile,bucket) slot targets, multiples of 128.
    95 cells @512 + 3 cells @640 per bucket, large cells spread over tiles."""
    T = np.full((NTILE, NCHUNK), 512, np.int64)
    for b in range(NCHUNK):
        for j in range(3):
            T[(b * 3 + j) % NTILE, b] = 640
    return T


def _balance_tiles(deg, T):
    """deg: [NL, NCHUNK] per-node per-bucket in-degree. Pack nodes into 98
    tiles (<=128 nodes) aiming to keep per-(tile,bucket) sums under the
    shared targets T."""
    order = np.argsort(-deg.sum(1), kind="stable")
    sums = np.zeros((NTILE, NCHUNK), np.int64)
    cap = np.full(NTILE, 128, np.int64)
    tile_of = np.empty(NL, np.int64)
    Tf = T.astype(np.float64)
    for n in order:
        d = deg[n]
        if d.sum() == 0:
            t = int(np.argmax(cap))
        else:
            ns = sums + d
            over = np.maximum(ns - T, 0).sum(axis=1).astype(np.float64)
            frac = (ns / Tf).max(axis=1)
            score = over * 1e3 + frac
            score[cap <= 0] = np.inf
            t = int(np.argmin(score))
        tile_of[n] = t
        sums[t] += d
        cap[t] -= 1
    return tile_of, sums


def _prep(inputs):
    x = np.asarray(inputs["x"], np.float32)
    ei = np.asarray(inputs["edge_index"])
    ea = np.asarray(inputs["edge_attr"], np.float32)
    batch = np.asarray(inputs["batch"]).astype(np.int64)
    src, dst_g = ei[0].astype(np.int64), ei[1].astype(np.int64)

    owner = dst_g // NL
    b_edge = (src // NL) // 2          # == pad_id // CHUNK, permutation-free

    # --- balanced tile assignment per core ---
    tile_of_g = np.empty(N_NODES, np.int64)
    pos_g = np.empty(N_NODES, np.int64)     # local padded id within owner
    cnts = np.zeros((N_CORES, NCHUNK, NTILE), np.int64)
    per_core_sel = []
    Tgt = _cell_targets()
    for k in range(N_CORES):
        sel = np.nonzero(owner == k)[0]
        d_loc = dst_g[sel] - k * NL
        deg = np.zeros((NL, NCHUNK), np.int64)
        np.add.at(deg, (d_loc, b_edge[sel]), 1)
        tile_of, sums = _balance_tiles(deg, Tgt)
        cnts[k] = sums.T                     # [NCHUNK, NTILE]
        # positions within tile: stable by node id
        pos_in = np.zeros(NL, np.int64)
        for t in range(NTILE):
            nodes = np.nonzero(tile_of == t)[0]
            pos_in[nodes] = np.arange(len(nodes))
        tile_of_g[k * NL:(k + 1) * NL] = tile_of
        pos_g[k * NL:(k + 1) * NL] = tile_of * 128 + pos_in
        per_core_sel.append((sel, d_loc))

    pid_g = (np.arange(N_NODES) // NL) * NLP + pos_g   # padded global row

    ngroups = np.ceil(cnts / 128.0).astype(np.int64).max(axis=0)  # [NCHUNK, NTILE]
    schedule = []        # (b, t, off, gw)
    off = 0
    for b in range(NCHUNK):
        for t in range(NTILE):
            g = int(ngroups[b, t])
            if g == 0:
                continue
            schedule.append((b, t, off, g * 128))
            off += g * 128
    nslot = off
    maxgw = max(s[3] for s in schedule)
    assert maxgw <= 768, maxgw

    # gather batches: cell-aligned, <= GB slots, single bucket
    batches = []         # (off, n, bkt)
    cur_off, cur_n, cur_b = None, 0, None
    for (b, t, o, gw) in schedule:
        if cur_off is not None and (b != cur_b or cur_n + gw > GB):
            batches.append((cur_off, cur_n, cur_b))
            cur_off = None
        if cur_off is None:
            cur_off, cur_n, cur_b = o, 0, b
        cur_n += gw
    if cur_off is not None:
        batches.append((cur_off, cur_n, cur_b))

    # --- weights ---
    Wf1 = np.asarray(inputs["Wf1"], np.float32); bf1 = np.asarray(inputs["bf1"], np.float32)
    Ws1 = np.asarray(inputs["Ws1"], np.float32); bs1 = np.asarray(inputs["bs1"], np.float32)
    Wp = np.asarray(inputs["Wp"], np.float32); bp = np.asarray(inputs["bp"], np.float32)
    P = {nm: np.asarray(inputs[nm], np.float32) for nm in
         ["Wf2", "bf2", "Ws2", "bs2", "Wf3", "bf3", "Ws3", "bs3", "W1", "b1", "W2", "b2"]}

    def cw(W, b):
        ws = W[HIDDEN:2 * HIDDEN]                     # src part  [128,128]
        we = np.zeros((33, HIDDEN), np.float32)
        we[:32] = W[2 * HIDDEN:]
        we[32] = b
        wd = W[:HIDDEN]                               # dst part (rhs for Y)
        return ws.astype(bf16), we.astype(bf16), wd.astype(bf16)

    w2s, w2e, w2da = cw(P["Wf2"], P["bf2"]); s2s, s2e, s2da = cw(P["Ws2"], P["bs2"])
    w3s, w3e, w3da = cw(P["Wf3"], P["bf3"]); s3s, s3e, s3da = cw(P["Ws3"], P["bs3"])
    w2d = np.concatenate([w2da, s2da], axis=1)        # [128, 256]
    w3d = np.concatenate([w3da, s3da], axis=1)

    def c1w(W, b):
        ws = np.zeros((128, 4), np.float32); ws[0:3, :3] = W[3:6]
        we = np.zeros((33, 4), np.float32); we[:32, :3] = W[6:]; we[32, :3] = b
        wd = np.zeros((4, 4), np.float32); wd[0:3, :3] = W[0:3]
        return ws.astype(bf16), we.astype(bf16), wd.astype(bf16)

    w1s, w1e, w1d = c1w(Wf1, bf1); s1s, s1e, s1d = c1w(Ws1, bs1)
    wp_aug = np.zeros((4, HIDDEN), np.float32); wp_aug[:3] = Wp; wp_aug[3] = bp

    gcnts = np.bincount(batch, minlength=N_GRAPHS).astype(np.float32)
    inv_cnt = (1.0 / np.maximum(gcnts, 1.0)).reshape(N_GRAPHS, 1)

    identb = np.eye(128, dtype=np.float32).astype(bf16)
    identf = np.eye(128, dtype=np.float32)

    xpad = np.zeros((NFULL, 128), bf16)
    xpad[pid_g, :3] = x.astype(bf16)

    in_maps = []
    for k in range(N_CORES):
        sel, d_loc = per_core_sel[k]
        dslot = pos_g[k * NL:(k + 1) * NL][d_loc]          # padded local id of dst
        d_tile = dslot // 128
        d_in_tile = dslot % 128
        eorder = np.lexsort((d_in_tile, d_tile, b_edge[sel]))
        es, dt_s, dit_s, bb = (sel[eorder], d_tile[eorder],
                               d_in_tile[eorder], b_edge[sel][eorder])

        gsrc = np.zeros(nslot, np.int16)
        eaT = np.zeros((33, nslot), np.float32)
        ohblk = np.zeros((128, 2 * nslot), bf16)
        ptr = 0
        n_e = len(es)
        for (b, t, o, gw) in schedule:
            p2 = ptr
            while p2 < n_e and bb[p2] == b and dt_s[p2] == t:
                p2 += 1
            cnt = p2 - ptr
            assert cnt <= gw
            gsrc[o:o + cnt] = (pid_g[src[es[ptr:p2]]] % CHUNK).astype(np.int16)
            eaT[:32, o:o + cnt] = ea[es[ptr:p2]].T
            eaT[32, o:o + cnt] = 1.0
            dd = dit_s[ptr:p2]
            # oh2 [node_slot, edge]
            ohblk[dd, 2 * o + np.arange(cnt)] = 1.0
            # oh per group [edge_in_group, node_slot]; 0.5 folds sigmoid's /2
            jj = np.arange(cnt)
            ohblk[jj % 128, 2 * o + gw + (jj // 128) * 128 + dd] = 0.5
            ptr = p2
        assert ptr == n_e

        xT_sb = np.zeros((4, NLP), np.float32)
        xl = x[k * NL:(k + 1) * NL]
        pidl = pos_g[k * NL:(k + 1) * NL]
        xT_sb[:3, pidl] = xl.T

        xrow = np.zeros((128, NTILE * 4), np.float32)
        xrow[pidl % 128, (pidl // 128) * 4 + 0] = xl[:, 0]
        xrow[pidl % 128, (pidl // 128) * 4 + 1] = xl[:, 1]
        xrow[pidl % 128, (pidl // 128) * 4 + 2] = xl[:, 2]
        xrow[:, 3::4] = 1.0

        ind = np.zeros((NLP, N_GRAPHS), np.float32)
        ind[pidl, batch[k * NL:(k + 1) * NL]] = 1.0

        in_maps.append(dict(
            xpad=xpad,
            gsrc=_wrap16(gsrc),
            eaT=eaT.astype(bf16),
            ohblk=ohblk,
            xT_sb=xT_sb.astype(bf16),
            xrow=xrow,
            identb=identb, identf=identf,
            ind=ind.reshape(NTILE, 128, N_GRAPHS).transpose(1, 0, 2)
                  .reshape(128, NTILE * N_GRAPHS).copy(),
            inv_cnt=inv_cnt,
            w1s=w1s, w1e=w1e, w1d=w1d, s1s=s1s, s1e=s1e, s1d=s1d,
            w2s=w2s, w2e=w2e, w2d=w2d, s2s=s2s, s2e=s2e,
            w3s=w3s, w3e=w3e, w3d=w3d, s3s=s3s, s3e=s3e,
            wp_aug=wp_aug.astype(bf16),
            hw1=P["W1"], hb1=P["b1"].reshape(1, HIDDEN).copy(),
            hw2=P["W2"], hb2=P["b2"].reshape(1, OUT_DIM).copy(),
        ))
    return in_maps, schedule, batches, nslot, maxgw


def _conv_pass(nc, cdim, table_d, gidx_t, eaT_d, ohblk_d, y_src, wfs, wfe,
               wss, wse, id_t, schedule, batches, agg, pools):
    """y_src: ('sbuf', tile) with [128, NTILE*8] for conv1 (f cols 0:4, s 4:8)
    or ('hbm', dram) with [128, NTILE*256] for conv2/3 (f 0:128, s 128:256)."""
    pool, psum_fs, psum_m, psum_ag, gpool, ipool = pools
    cp = 4 if cdim == 3 else 128
    acols = 4 if cdim == 3 else 128
    A = mybir.ActivationFunctionType

    # map cell -> batch
    cell_batch = {}
    for bi, (boff, bn, bkt) in enumerate(batches):
        for ci, (b, t, o, gw) in enumerate(schedule):
            if boff <= o < boff + bn:
                cell_batch[ci] = bi

    def gather(bi):
        boff, bn, bkt = batches[bi]
        hb = gpool.tile([128, GB], dt.bfloat16, tag="hsrc")
        step = bn if bn <= 768 else GCALL
        for j in range(0, bn, step):
            n = min(step, bn - j)
            nc.gpsimd.dma_gather(
                out_ap=hb[:, j:j + n].rearrange("p (g e) -> p g e", g=1),
                in_ap=table_d[bkt * CHUNK:(bkt + 1) * CHUNK, :],
                idxs_ap=gidx_t[:, (boff + j) // 16:(boff + j + n) // 16],
                num_idxs=n, num_idxs_reg=n, elem_size=128, transpose=True)
        return hb

    pending = {}
    pending[0] = gather(0)
    if len(batches) > 1:
        pending[1] = gather(1)
    cur_bi = 0
    hb = pending.pop(0)
    for ci, (b, t, o, gw) in enumerate(schedule):
        bi = cell_batch[ci]
        if bi != cur_bi:
            hb = pending.pop(bi) if bi in pending else gather(bi)
            cur_bi = bi
        for pf in (bi + 1, bi + 2, bi + 3, bi + 4):
            if pf < len(batches) and pf not in pending and pf > bi:
                if all(p <= bi or p in pending for p in range(bi + 1, pf)):
                    if pf - bi <= 4 and pf not in pending:
                        pending[pf] = gather(pf)

        boff = batches[bi][0]
        oh_t = ipool.tile([128, 2 * 768], dt.bfloat16, tag="oh")
        nc.sync.dma_start(out=oh_t[:, :2 * gw], in_=ohblk_d[:, 2 * o:2 * o + 2 * gw])
        ea_t = ipool.tile([33, 768], dt.bfloat16, tag="ea")
        nc.sync.dma_start(out=ea_t[:, :gw], in_=eaT_d[:, o:o + gw])
        if y_src[0] == "hbm":
            y_t = ipool.tile([128, 256], dt.bfloat16, tag="yy")
            nc.sync.dma_start(out=y_t[:], in_=y_src[1][:, t * 256:(t + 1) * 256])
            yf, ys = y_t[:, 0:cdim], y_t[:, 128:128 + cdim]
        else:
            y_t = y_src[1]
            yf, ys = y_t[:, t * 8:t * 8 + 4], y_t[:, t * 8 + 4:t * 8 + 8]

        ng = gw // 128
        ag = psum_ag.tile([128, acols], dt.float32, space="PSUM", tag="ag",
                          name=f"ag{ci}")
        gg = 0
        for c0 in range(0, ng, 4):
            cn = min(4, ng - c0)
            cw_ = cn * 128
            co = c0 * 128
            f_ps = psum_fs.tile([128, 512], dt.float32, space="PSUM", tag="f")
            s_ps = psum_fs.tile([128, 512], dt.float32, space="PSUM", tag="s")
            hs = hb[:, o - boff + co:o - boff + co + cw_]
            eb = ea_t[:33, co:co + cw_]
            o2 = oh_t[:, co:co + cw_]
            nc.tensor.matmul(f_ps[:cp, :cw_], lhsT=wfs[:], rhs=hs, start=True, stop=False)
            nc.tensor.matmul(f_ps[:cp, :cw_], lhsT=wfe[:], rhs=eb, start=False, stop=False)
            nc.tensor.matmul(f_ps[:cp, :cw_], lhsT=yf, rhs=o2, start=False, stop=True)
            nc.tensor.matmul(s_ps[:cp, :cw_], lhsT=wss[:], rhs=hs, start=True, stop=False)
            nc.tensor.matmul(s_ps[:cp, :cw_], lhsT=wse[:], rhs=eb, start=False, stop=False)
            nc.tensor.matmul(s_ps[:cp, :cw_], lhsT=ys, rhs=o2, start=False, stop=True)

            # m' = (tanh(F/2)+1) * (silu(S) + A - A*tanh^2(B*S));  the /2 of
            # sigmoid is folded into the one-hot aggregation values (0.5)
            O = mybir.AluOpType
            tf = pool.tile([128, 512], dt.bfloat16, tag="tf")
            ss = pool.tile([128, 512], dt.bfloat16, tag="ss")
            ts = pool.tile([128, 512], dt.bfloat16, tag="ts")
            nc.scalar.activation(tf[:cp, :cw_], f_ps[:cp, :cw_], A.Tanh, scale=0.5)
            nc.scalar.activation(ss[:cp, :cw_], s_ps[:cp, :cw_], A.Silu)
            nc.scalar.activation(ts[:cp, :cw_], s_ps[:cp, :cw_], A.Tanh, scale=B_SP)
            t1 = pool.tile([128, 512], dt.bfloat16, tag="t1")
            nc.vector.scalar_tensor_tensor(out=t1[:cp, :cw_], in0=ts[:cp, :cw_],
                                           scalar=-A_SP, in1=ts[:cp, :cw_],
                                           op0=O.mult, op1=O.mult)
            t2 = pool.tile([128, 512], dt.bfloat16, tag="t2")
            nc.vector.scalar_tensor_tensor(out=t2[:cp, :cw_], in0=t1[:cp, :cw_],
                                           scalar=A_SP, in1=ss[:cp, :cw_],
                                           op0=O.add, op1=O.add)
            m_bf = pool.tile([128, 512], dt.bfloat16, tag="mbf")
            nc.vector.scalar_tensor_tensor(out=m_bf[:cp, :cw_], in0=tf[:cp, :cw_],
                                           scalar=1.0, in1=t2[:cp, :cw_],
                                           op0=O.add, op1=O.mult)

            for g in range(cn):
                m_ps = psum_m.tile([128, 128], dt.bfloat16, space="PSUM", tag="mt")
                nc.tensor.transpose(m_ps[:, :cdim], m_bf[:cdim, g * 128:(g + 1) * 128],
                                    id_t[:cdim, :cdim])
                m_sb = pool.tile([128, 128], dt.bfloat16, tag="msb")
                if (gg % 2) == 0:
                    nc.vector.tensor_copy(out=m_sb[:, :cdim], in_=m_ps[:, :cdim])
                else:
                    nc.scalar.copy(out=m_sb[:, :cdim], in_=m_ps[:, :cdim])
                ohg = oh_t[:, gw + (gg * 128):gw + (gg + 1) * 128]
                nc.tensor.matmul(ag[:, :cdim], lhsT=ohg, rhs=m_sb[:, :cdim],
                                 start=(gg == 0), stop=(gg == ng - 1))
                gg += 1
        nc.vector.tensor_add(out=agg[:, t * acols:t * acols + cdim],
                             in0=agg[:, t * acols:t * acols + cdim],
                             in1=ag[:, :cdim])


def build(schedule, batches, nslot, maxgw):
    nc = bacc.Bacc("TRN2", target_bir_lowering=False, debug=False, num_devices=N_CORES)
    D = {}

    def din(name, shape, dtype):
        D[name] = nc.dram_tensor(name, list(shape), dtype, kind="ExternalInput")
        return D[name]

    xpad_d = din("xpad", (NFULL, 128), dt.bfloat16)
    gsrc_d = din("gsrc", (128, nslot // 16), dt.int16)
    eaT_d = din("eaT", (33, nslot), dt.bfloat16)
    ohblk_d = din("ohblk", (128, 2 * nslot), dt.bfloat16)
    xT_d = din("xT_sb", (4, NLP), dt.bfloat16)
    xrow_d = din("xrow", (128, NTILE * 4), dt.float32)
    identb_d = din("identb", (128, 128), dt.bfloat16)
    identf_d = din("identf", (128, 128), dt.float32)
    ind_d = din("ind", (128, NTILE * N_GRAPHS), dt.float32)
    invc_d = din("inv_cnt", (N_GRAPHS, 1), dt.float32)
    wshapes = [("w1s", (128, 4)), ("w1e", (33, 4)), ("w1d", (4, 4)),
               ("s1s", (128, 4)), ("s1e", (33, 4)), ("s1d", (4, 4)),
               ("w2s", (128, 128)), ("w2e", (33, 128)), ("w2d", (128, 256)),
               ("s2s", (128, 128)), ("s2e", (33, 128)),
               ("w3s", (128, 128)), ("w3e", (33, 128)), ("w3d", (128, 256)),
               ("s3s", (128, 128)), ("s3e", (33, 128)),
               ("wp_aug", (4, 128))]
    for nm, sh in wshapes:
        din(nm, sh, dt.bfloat16)
    hw1_d = din("hw1", (HIDDEN, HIDDEN), dt.float32)
    hb1_d = din("hb1", (1, HIDDEN), dt.float32)
    hw2_d = din("hw2", (HIDDEN, OUT_DIM), dt.float32)
    hb2_d = din("hb2", (1, OUT_DIM), dt.float32)

    out_d = nc.dram_tensor("out", [N_GRAPHS, OUT_DIM], dt.float32, kind="ExternalOutput")

    h_local = nc.dram_tensor("h_local", [NLP, 128], dt.bfloat16)
    h_full = nc.dram_tensor("h_full", [NFULL, 128], dt.bfloat16, addr_space="Shared")
    h2_local = nc.dram_tensor("h2_local", [NLP, 128], dt.bfloat16)
    h2_full = nc.dram_tensor("h2_full", [NFULL, 128], dt.bfloat16, addr_space="Shared")
    y_hbm = nc.dram_tensor("y_hbm", [128, NTILE * 256], dt.bfloat16)
    pool_in = nc.dram_tensor("pool_in", [N_GRAPHS, HIDDEN], dt.float32)
    pool_out = nc.dram_tensor("pool_out", [N_GRAPHS, HIDDEN], dt.float32,
                              addr_space="Shared")

    O = mybir.AluOpType
    A = mybir.ActivationFunctionType

    with tile.TileContext(nc, num_cores=N_CORES) as tc:
        with (
            tc.tile_pool(name="const", bufs=1) as cpool,
            tc.tile_pool(name="work", bufs=3) as pool,
            tc.tile_pool(name="gath", bufs=8) as gpool,
            tc.tile_pool(name="io", bufs=2) as ipool,
            tc.tile_pool(name="psfs", bufs=2, space="PSUM") as psum_fs,
            tc.tile_pool(name="psm", bufs=2, space="PSUM") as psum_m,
            tc.tile_pool(name="psag", bufs=1, space="PSUM") as psum_ag,
            tc.tile_pool(name="psy", bufs=1, space="PSUM") as psum_y,
        ):
            W = {}
            for nm, sh in wshapes:
                W[nm] = cpool.tile(list(sh), dt.bfloat16, tag=nm, name=f"w_{nm}")
                nc.sync.dma_start(out=W[nm][:], in_=D[nm][:])
            id_t = cpool.tile([128, 128], dt.bfloat16, tag="idt")
            idf_t = cpool.tile([128, 128], dt.float32, tag="idf")
            nc.sync.dma_start(out=id_t[:], in_=identb_d[:])
            nc.sync.dma_start(out=idf_t[:], in_=identf_d[:])
            gidx_t = cpool.tile([128, nslot // 16], dt.int16, tag="gidx")
            nc.sync.dma_start(out=gidx_t[:], in_=gsrc_d[:])
            xT_t = cpool.tile([4, NLP], dt.bfloat16, tag="xT")
            nc.sync.dma_start(out=xT_t[:], in_=xT_d[:])

            hT = cpool.tile([128, NLP], dt.bfloat16, tag="hT")

            pools = (pool, psum_fs, psum_m, psum_ag, gpool, ipool)

            # ---- conv1 Y (from xT) ----
            y1 = cpool.tile([128, NTILE * 8], dt.bfloat16, tag="y1")
            for t in range(NTILE):
                yp = psum_y.tile([128, 256], dt.float32, space="PSUM", tag="yf")
                nc.tensor.matmul(yp[:, 0:4], lhsT=xT_t[:, t * 128:(t + 1) * 128],
                                 rhs=W["w1d"][:], start=True, stop=True)
                nc.tensor.matmul(yp[:, 4:8], lhsT=xT_t[:, t * 128:(t + 1) * 128],
                                 rhs=W["s1d"][:], start=True, stop=True)
                nc.vector.tensor_copy(out=y1[:, t * 8:(t + 1) * 8], in_=yp[:, :8])

            # ---- conv1 ----
            agg1 = cpool.tile([128, NTILE * 4], dt.float32, tag="agg1")
            nc.vector.memset(agg1[:], 0.0)
            _conv_pass(nc, NODE_DIM, xpad_d, gidx_t, eaT_d, ohblk_d,
                       ("sbuf", y1), W["w1s"], W["w1e"], W["s1s"], W["s1e"],
                       id_t, schedule, batches, agg1, pools)

            # ---- lift: h = relu((x + agg1) @ Wp + bp), build hT ----
            xr = cpool.tile([128, NTILE * 4], dt.float32, tag="xr")
            nc.sync.dma_start(out=xr[:], in_=xrow_d[:])
            h0 = cpool.tile([128, NTILE * 4], dt.float32, tag="h0")
            nc.vector.tensor_add(out=h0[:], in0=xr[:], in1=agg1[:])
            for t in range(NTILE):
                h0t_ps = psum_y.tile([128, 256], dt.float32, space="PSUM", tag="yf")
                nc.tensor.transpose(h0t_ps[:4, :128], h0[:, t * 4:(t + 1) * 4], idf_t[:])
                h0aug = pool.tile([4, 128], dt.bfloat16, tag="h0aug")
                nc.vector.tensor_copy(out=h0aug[:, :], in_=h0t_ps[:4, :128])
                hl_ps = psum_y.tile([128, 256], dt.float32, space="PSUM", tag="yf")
                nc.tensor.matmul(hl_ps[:, :128], lhsT=h0aug[:], rhs=W["wp_aug"][:],
                                 start=True, stop=True)
                h_sb = pool.tile([128, 128], dt.bfloat16, tag="hsb")
                nc.scalar.activation(h_sb[:], hl_ps[:, :128], A.Relu)
                nc.sync.dma_start(out=h_local[t * 128:(t + 1) * 128, :], in_=h_sb[:])
                ht_ps = psum_m.tile([128, 128], dt.bfloat16, space="PSUM", tag="mt")
                nc.tensor.transpose(ht_ps[:], h_sb[:], id_t[:])
                nc.scalar.copy(out=hT[:, t * 128:(t + 1) * 128], in_=ht_ps[:])

            nc.gpsimd.collective_compute(
                "AllGather", O.bypass, replica_groups=[list(range(N_CORES))],
                ins=[h_local[:]], outs=[h_full[:]])

            def conv_hidden(wd_fs, wfs, wfe, wss, wse, table_full, aggH):
                # Y phase: y_hbm[:, t*256:...] = h_tile @ [Wfd | Wsd]
                for t in range(NTILE):
                    yp = psum_y.tile([128, 256], dt.float32, space="PSUM", tag="yf")
                    nc.tensor.matmul(yp[:, 0:256], lhsT=hT[:, t * 128:(t + 1) * 128],
                                     rhs=wd_fs[:], start=True, stop=True)
                    y_sb = pool.tile([128, 256], dt.bfloat16, tag="ysb")
                    nc.scalar.copy(out=y_sb[:], in_=yp[:])
                    nc.sync.dma_start(out=y_hbm[:, t * 256:(t + 1) * 256], in_=y_sb[:])
                nc.vector.memset(aggH[:], 0.0)
                _conv_pass(nc, HIDDEN, table_full, gidx_t, eaT_d, ohblk_d,
                           ("hbm", y_hbm), wfs, wfe, wss, wse,
                           id_t, schedule, batches, aggH, pools)

            # ---- conv2 ----
            aggH = cpool.tile([128, NTILE * 128], dt.float32, tag="aggH")
            conv_hidden(W["w2d"], W["w2s"], W["w2e"], W["s2s"], W["s2e"],
                        h_full, aggH)

            # update h2 = relu(h + aggH); write h2_local + hT
            for t in range(NTILE):
                hprev = ipool.tile([128, 128], dt.bfloat16, tag="hprev")
                nc.sync.dma_start(out=hprev[:], in_=h_local[t * 128:(t + 1) * 128, :])
                h2_sb = pool.tile([128, 128], dt.bfloat16, tag="h2sb")
                nc.vector.tensor_add(out=h2_sb[:], in0=aggH[:, t * 128:(t + 1) * 128],
                                     in1=hprev[:])
                nc.vector.tensor_scalar_max(out=h2_sb[:], in0=h2_sb[:], scalar1=0.0)
                nc.sync.dma_start(out=h2_local[t * 128:(t + 1) * 128, :], in_=h2_sb[:])
                ht_ps = psum_m.tile([128, 128], dt.bfloat16, space="PSUM", tag="mt")
                nc.tensor.transpose(ht_ps[:], h2_sb[:], id_t[:])
                nc.scalar.copy(out=hT[:, t * 128:(t + 1) * 128], in_=ht_ps[:])

            nc.gpsimd.collective_compute(
                "AllGather", O.bypass, replica_groups=[list(range(N_CORES))],
                ins=[h2_local[:]], outs=[h2_full[:]])

            # ---- conv3 ----
            agg3 = cpool.tile([128, NTILE * 128], dt.float32, tag="aggH")
            conv_hidden(W["w3d"], W["w3s"], W["w3e"], W["s3s"], W["s3e"],
                        h2_full, agg3)

            # ---- h3 = relu(h2 + agg3); pooling ----
            pl_full = psum_ag.tile([128, HIDDEN], dt.float32, space="PSUM", tag="ag")
            pl_ps = pl_full[:N_GRAPHS, :]
            for t in range(NTILE):
                hprev = ipool.tile([128, 128], dt.bfloat16, tag="hprev")
                nc.sync.dma_start(out=hprev[:], in_=h2_local[t * 128:(t + 1) * 128, :])
                indt = ipool.tile([128, N_GRAPHS], dt.float32, tag="indt")
                nc.sync.dma_start(out=indt[:],
                                  in_=ind_d[:, t * N_GRAPHS:(t + 1) * N_GRAPHS])
                indb = pool.tile([128, N_GRAPHS], dt.bfloat16, tag="indb")
                nc.vector.tensor_copy(out=indb[:], in_=indt[:])
                h3_sb = pool.tile([128, 128], dt.bfloat16, tag="h2sb")
                nc.vector.tensor_add(out=h3_sb[:], in0=agg3[:, t * 128:(t + 1) * 128],
                                     in1=hprev[:])
                nc.vector.tensor_scalar_max(out=h3_sb[:], in0=h3_sb[:], scalar1=0.0)
                nc.tensor.matmul(pl_ps, lhsT=indb[:], rhs=h3_sb[:],
                                 start=(t == 0), stop=(t == NTILE - 1))

            pl_sb = cpool.tile([N_GRAPHS, HIDDEN], dt.float32, tag="plsb")
            nc.vector.tensor_copy(out=pl_sb[:], in_=pl_ps)
            nc.sync.dma_start(out=pool_in[:], in_=pl_sb[:])
            nc.gpsimd.collective_compute(
                "AllReduce", O.add, replica_groups=[list(range(N_CORES))],
                ins=[pool_in[:]], outs=[pool_out[:]])

            # ---- head ----
            invc_t = cpool.tile([N_GRAPHS, 1], dt.float32, tag="invc")
            nc.sync.dma_start(out=invc_t[:], in_=invc_d[:])
            pooled = cpool.tile([N_GRAPHS, HIDDEN], dt.float32, tag="pooled")
            nc.sync.dma_start(out=pooled[:], in_=pool_out[:])
            nc.vector.tensor_scalar(out=pooled[:], in0=pooled[:],
                                    scalar1=invc_t[:, 0:1], scalar2=None, op0=O.mult)
            w1_t = cpool.tile([HIDDEN, HIDDEN], dt.float32, tag="w1")
            b1_t = cpool.tile([1, HIDDEN], dt.float32, tag="b1")
            w2_t = cpool.tile([HIDDEN, OUT_DIM], dt.float32, tag="w2")
            b2_t = cpool.tile([1, OUT_DIM], dt.float32, tag="b2")
            ones_g = cpool.tile([1, N_GRAPHS], dt.float32, tag="onesg")
            nc.vector.memset(ones_g[:], 1.0)
            for d_, s_ in [(w1_t, hw1_d), (b1_t, hb1_d), (w2_t, hw2_d), (b2_t, hb2_d)]:
                nc.sync.dma_start(out=d_[:], in_=s_[:])

            ptp = psum_y.tile([128, 256], dt.float32, space="PSUM", tag="yf")
            nc.tensor.transpose(ptp[:, :N_GRAPHS], pooled[:], idf_t[:N_GRAPHS, :N_GRAPHS])
            pooledT = cpool.tile([HIDDEN, N_GRAPHS], dt.float32, tag="pT")
            nc.vector.tensor_copy(out=pooledT[:], in_=ptp[:, :N_GRAPHS])
            hh_ps = psum_y.tile([128, 256], dt.float32, space="PSUM", tag="yf")
            nc.tensor.matmul(hh_ps[:N_GRAPHS, :128], lhsT=pooledT[:], rhs=w1_t[:],
                             start=True, stop=False)
            nc.tensor.matmul(hh_ps[:N_GRAPHS, :128], lhsT=ones_g[:], rhs=b1_t[:],
                             start=False, stop=True)
            hh = cpool.tile([N_GRAPHS, HIDDEN], dt.float32, tag="hh")
            nc.scalar.activation(hh[:], hh_ps[:N_GRAPHS, :128], A.Relu)
            htp = psum_y.tile([128, 256], dt.float32, space="PSUM", tag="yf")
            nc.tensor.transpose(htp[:, :N_GRAPHS], hh[:], idf_t[:N_GRAPHS, :N_GRAPHS])
            hhT = cpool.tile([HIDDEN, N_GRAPHS], dt.float32, tag="hhT")
            nc.vector.tensor_copy(out=hhT[:], in_=htp[:, :N_GRAPHS])
            out_ps = psum_y.tile([128, 256], dt.float32, space="PSUM", tag="yf")
            nc.tensor.matmul(out_ps[:N_GRAPHS, :OUT_DIM], lhsT=hhT[:], rhs=w2_t[:],
                             start=True, stop=False)
            nc.tensor.matmul(out_ps[:N_GRAPHS, :OUT_DIM], lhsT=ones_g[:], rhs=b2_t[:],
                             start=False, stop=True)
            out_sb = cpool.tile([N_GRAPHS, OUT_DIM], dt.float32, tag="osb")
            nc.vector.tensor_copy(out=out_sb[:], in_=out_ps[:N_GRAPHS, :OUT_DIM])
            nc.sync.dma_start(out=out_d[:], in_=out_sb[:])

    nc.compile()
    return nc


def kernel(**inputs) -> np.ndarray:
    in_maps, schedule, batches, nslot, maxgw = _prep(inputs)
    nc = build(schedule, batches, nslot, maxgw)
    res = run_bass_kernel_spmd(nc, in_maps, list(range(N_CORES)))
    return res.results[0]["out"].astype(np.float32)
